# revision 3
# baseline (speedup 1.0000x reference)
"""GQA attention block on 8 Trainium2 cores — all-bf16 matmul pipeline.

Sharding: data-parallel over batch B=2 x tensor-parallel over the 4 KV groups
(cores 0-3 -> batch 0 groups 0-3, cores 4-7 -> batch 1 groups 0-3).
Each core computes Q/K/V projections for its group, attention for its 4 query
heads, and a row-sharded partial of the output projection.  The host sums the
4 partials per batch (fp32) and adds the output bias.

v2 changes vs the fp32r baseline:
  - every matmul operand is bf16 (fp32 PSUM accumulation).  bf16 stationary
    weights get Fast Weight Load; the fp32r baseline measured 324 ns per
    128x128x512 matmul vs the 216 ns warm roofline, most of it the 4-byte
    weight-load path.
  - Q/K/V bias-adds moved from Scalar(ACT) to Vector (tensor_scalar_add) so
    ACT does nothing but the softmax EXPs (hard floor (N+352)/1.2 ns each).
  - softmax denominator partial sums split Vector/GpSimd, recip broadcast on
    GpSimd (partition_broadcast) instead of a PE ones-matmul + ACT copy.
  - softmax tails and the out-projection of chunk sc are spread as filler
    into the ACT-paced slots of the next chunk's score/attnV loops, keeping
    the PE stream dense.
  - OUT partials are bf16 (halves output DMA); host accumulates in fp32.
"""
import sys

sys.path.insert(0, "/opt/trn_rl_repo")

import math
from contextlib import ExitStack

import numpy as np
import ml_dtypes

import concourse.bacc as bacc
import concourse.tile as tile
import concourse.mybir as mybir
from concourse.bass_utils import run_bass_kernel_spmd
from concourse.masks import make_identity

F32 = mybir.dt.float32
BF16 = mybir.dt.bfloat16
AF = mybir.ActivationFunctionType
NPBF16 = ml_dtypes.bfloat16

D = 2048          # d_model
S = 2048          # sequence length
HD = 128          # head dim
R = 4             # q heads per kv group (on one core)
GD = R * HD       # 512: q-projection width per core
KT_TILES = S // 128   # 16 key-time tiles
KD_TILES = D // 128   # 16 contraction tiles for projections
N_SC = 4          # s-chunks of 512
SC = S // N_SC    # 512
SCALE = 1.0 / math.sqrt(HD)

_CACHED = {}


def _build():
    nc = bacc.Bacc("TRN2", target_bir_lowering=False, debug=False, num_devices=8)

    XT = nc.dram_tensor("xt", [D, S], BF16, kind="ExternalInput")
    WQ = nc.dram_tensor("wq", [D, GD], BF16, kind="ExternalInput")
    WK = nc.dram_tensor("wk", [D, HD], BF16, kind="ExternalInput")
    WV = nc.dram_tensor("wv", [D, HD], BF16, kind="ExternalInput")
    WO = nc.dram_tensor("wo", [GD, D], BF16, kind="ExternalInput")
    BQ = nc.dram_tensor("bq", [128, R], F32, kind="ExternalInput")
    BK = nc.dram_tensor("bk", [128, 1], F32, kind="ExternalInput")
    BV = nc.dram_tensor("bv", [128, 1], F32, kind="ExternalInput")
    OUT = nc.dram_tensor("out", [S, D], BF16, kind="ExternalOutput")

    with tile.TileContext(nc) as tc, ExitStack() as ctx:
        # ---- long-lived tiles ----
        lp = ctx.enter_context(tc.tile_pool(name="long", bufs=1))
        qt_sb = lp.tile([128, R, S], BF16)        # Q^T per head: [dq, h, s]
        kt_sb = lp.tile([128, S], BF16)           # K^T: [dk, t]
        vt_sb = lp.tile([128, S], BF16)           # V^T: [dv, t]
        v_sb = lp.tile([128, KT_TILES, HD], BF16) # V natural: [t_sub, t_tile, dv]
        bq_sb = lp.tile([128, R], F32)
        bk_sb = lp.tile([128, 1], F32)
        bv_sb = lp.tile([128, 1], F32)
        ones_col = lp.tile([128, 1], BF16)
        ones_row = lp.tile([1, 128], BF16)
        ident = lp.tile([128, 128], BF16)

        nc.sync.dma_start(bq_sb[:], BQ.ap())
        nc.sync.dma_start(bk_sb[:], BK.ap())
        nc.sync.dma_start(bv_sb[:], BV.ap())

        nc.gpsimd.memset(ones_col[:], 1.0)
        nc.gpsimd.memset(ones_row[:], 1.0)
        make_identity(nc, ident[:])

        # ---- phase A: projections ----
        with ExitStack() as actx:
            wp = actx.enter_context(tc.tile_pool(name="wqkv", bufs=1))
            xp = actx.enter_context(tc.tile_pool(name="xt", bufs=2))
            psa = actx.enter_context(tc.tile_pool(name="psa", bufs=4, space="PSUM"))

            wq_sb = wp.tile([128, KD_TILES, GD], BF16)
            wk_sb = wp.tile([128, KD_TILES, HD], BF16)
            wv_sb = wp.tile([128, KD_TILES, HD], BF16)
            wq_r = WQ.ap().rearrange("(ko p) n -> p ko n", p=128)
            wk_r = WK.ap().rearrange("(ko p) n -> p ko n", p=128)
            wv_r = WV.ap().rearrange("(ko p) n -> p ko n", p=128)

            for sc in range(N_SC):
                xt = xp.tile([128, KD_TILES, SC], BF16, tag="xt")
                xt_r = XT.ap()[:, sc * SC:(sc + 1) * SC].rearrange(
                    "(ko p) s -> p ko s", p=128
                )
                # interleave per-k so the k=0 matmuls can start almost
                # immediately (weights ride along with the first chunk)
                for k in range(KD_TILES):
                    nc.sync.dma_start(xt[:, k, :], xt_r[:, k, :])
                    if sc == 0:
                        nc.sync.dma_start(wq_sb[:, k, :], wq_r[:, k, :])
                        nc.sync.dma_start(wk_sb[:, k, :], wk_r[:, k, :])
                        nc.sync.dma_start(wv_sb[:, k, :], wv_r[:, k, :])
                # Q^T for the 4 heads
                for dq in range(R):
                    ps = psa.tile([128, SC], F32, tag="psa")
                    for k in range(KD_TILES):
                        nc.tensor.matmul(
                            ps[:],
                            lhsT=wq_sb[:, k, dq * 128:(dq + 1) * 128],
                            rhs=xt[:, k, :],
                            start=(k == 0),
                            stop=(k == KD_TILES - 1),
                        )
                    nc.vector.tensor_scalar_add(
                        qt_sb[:, dq, sc * SC:(sc + 1) * SC], ps[:],
                        bq_sb[:, dq:dq + 1],
                    )
                # K^T
                ps = psa.tile([128, SC], F32, tag="psa")
                for k in range(KD_TILES):
                    nc.tensor.matmul(
                        ps[:], lhsT=wk_sb[:, k, :], rhs=xt[:, k, :],
                        start=(k == 0), stop=(k == KD_TILES - 1),
                    )
                nc.vector.tensor_scalar_add(
                    kt_sb[:, sc * SC:(sc + 1) * SC], ps[:], bk_sb[:],
                )
                # V^T
                ps = psa.tile([128, SC], F32, tag="psa")
                for k in range(KD_TILES):
                    nc.tensor.matmul(
                        ps[:], lhsT=wv_sb[:, k, :], rhs=xt[:, k, :],
                        start=(k == 0), stop=(k == KD_TILES - 1),
                    )
                nc.vector.tensor_scalar_add(
                    vt_sb[:, sc * SC:(sc + 1) * SC], ps[:], bv_sb[:],
                )

            # V^T -> V natural (16 PE transposes)
            pst = actx.enter_context(tc.tile_pool(name="pst", bufs=2, space="PSUM"))
            for t in range(KT_TILES):
                pt_ps = pst.tile([128, 128], BF16, tag="pst")
                nc.tensor.transpose(
                    pt_ps[:], vt_sb[:, t * 128:(t + 1) * 128], ident[:]
                )
                nc.vector.tensor_copy(v_sb[:, t, :], pt_ps[:])

        # ---- phase B: attention + out-proj ----
        with ExitStack() as bctx:
            wop = bctx.enter_context(tc.tile_pool(name="wo", bufs=1))
            wo_sb = wop.tile([128, R, D], BF16)
            nc.sync.dma_start(wo_sb[:], WO.ap().rearrange("(h p) n -> p h n", p=128))

            pss = bctx.enter_context(tc.tile_pool(name="pss", bufs=2, space="PSUM"))
            pso = bctx.enter_context(tc.tile_pool(name="pso", bufs=2, space="PSUM"))
            psm = bctx.enter_context(tc.tile_pool(name="psm", bufs=2, space="PSUM"))
            ptp = bctx.enter_context(tc.tile_pool(name="ptp", bufs=2))
            accp = bctx.enter_context(tc.tile_pool(name="accp", bufs=2))
            otp = bctx.enter_context(tc.tile_pool(name="otp", bufs=2))
            outp = bctx.enter_context(tc.tile_pool(name="outp", bufs=4))

            # filler queues consumed inside t_loop slots: tails have priority
            tail_q = []
            op_q = []

            def emit_filler(tg):
                if tg == 2 and tail_q:
                    tail_q.pop(0)()
                elif tg >= 2 and op_q:
                    op_q.pop(0)()

            def t_loop(sc, h):
                """scores -> exp -> attnV accumulation + split denom sums."""
                pt = ptp.tile([128, KT_TILES, SC], BF16, tag="pt", name="pt")
                accD = accp.tile([128, SC], BF16, tag="accD", name="accD")
                accG = accp.tile([128, SC], BF16, tag="accG", name="accG")
                ps_o = pso.tile([128, SC], F32, tag="pso", name="ps_o")
                q_ap = qt_sb[:, h, sc * SC:(sc + 1) * SC]

                def scores(tg):
                    ps_s = pss.tile([128, 2, SC], F32, tag="pss", name="ps_s")
                    for i in range(2):
                        t = tg * 2 + i
                        nc.tensor.matmul(
                            ps_s[:, i, :],
                            lhsT=kt_sb[:, t * 128:(t + 1) * 128],
                            rhs=q_ap,
                            start=True, stop=True,
                        )
                    return ps_s

                cur = scores(0)
                for tg in range(KT_TILES // 2):
                    nxt = scores(tg + 1) if tg < KT_TILES // 2 - 1 else None
                    nc.scalar.activation(
                        pt[:, 2 * tg:2 * tg + 2, :], cur[:], AF.Exp, scale=SCALE
                    )
                    for i in range(2):
                        t = tg * 2 + i
                        nc.tensor.matmul(
                            ps_o[:],
                            lhsT=v_sb[:, t, :],
                            rhs=pt[:, t, :],
                            start=(t == 0),
                            stop=(t == KT_TILES - 1),
                            skip_group_check=True,
                        )
                    eng = nc.vector if tg < 4 else nc.gpsimd
                    acc = accD if tg < 4 else accG
                    for i in range(2):
                        t = tg * 2 + i
                        if t in (0, 8):
                            eng.tensor_copy(acc[:], pt[:, t, :])
                        else:
                            eng.tensor_add(acc[:], acc[:], pt[:, t, :])
                    emit_filler(tg)
                    cur = nxt
                return ps_o, accD, accG

            def tail(sc, h, ot_sb, ps_o, accD, accG):
                """denominator -> reciprocal -> broadcast -> normalize."""
                acc_r = accp.tile([128, SC], BF16, tag="accr", name="acc_r")
                nc.vector.tensor_add(acc_r[:], accD[:], accG[:])
                ps_d = psm.tile([1, SC], F32, tag="psm", name="ps_d")
                nc.tensor.matmul(
                    ps_d[:], lhsT=ones_col[:], rhs=acc_r[:], start=True, stop=True
                )
                recip = accp.tile([1, SC], F32, tag="recip", name="recip")
                nc.vector.reciprocal_approx_fast(recip[:], ps_d[:])
                bc = accp.tile([128, SC], F32, tag="bc", name="bc")
                nc.gpsimd.partition_broadcast(bc[:], recip[:])
                nc.vector.tensor_mul(ot_sb[:, h, :], ps_o[:], bc[:])

            def queue_out_proj(sc, ot_sb):
                for st in range(SC // 128):
                    for oc in range(D // 512):
                        def go(st=st, oc=oc):
                            ps_f = psm.tile([128, 512], F32, tag="psm", name="ps_f")
                            for dv in range(R):
                                nc.tensor.matmul(
                                    ps_f[:],
                                    lhsT=ot_sb[:, dv, st * 128:(st + 1) * 128],
                                    rhs=wo_sb[:, dv, oc * 512:(oc + 1) * 512],
                                    start=(dv == 0),
                                    stop=(dv == R - 1),
                                    skip_group_check=True,
                                )
                            o_t = outp.tile([128, 512], BF16, tag="out", name="o_t")
                            if (st + oc) % 2 == 0:
                                nc.scalar.copy(o_t[:], ps_f[:])
                            else:
                                nc.vector.tensor_copy(o_t[:], ps_f[:])
                            nc.sync.dma_start(
                                OUT.ap()[
                                    sc * SC + st * 128: sc * SC + (st + 1) * 128,
                                    oc * 512:(oc + 1) * 512,
                                ],
                                o_t[:],
                            )
                        op_q.append(go)

            for sc in range(N_SC):
                ot_sb = otp.tile([128, R, SC], BF16, tag="ot", name="ot_sb")
                for h in range(R):
                    ps_o, accD, accG = t_loop(sc, h)
                    tail_q.append(
                        lambda sc=sc, h=h, ot_sb=ot_sb, ps_o=ps_o, accD=accD,
                        accG=accG: tail(sc, h, ot_sb, ps_o, accD, accG)
                    )
                queue_out_proj(sc, ot_sb)
            while tail_q:
                tail_q.pop(0)()
            while op_q:
                op_q.pop(0)()

    nc.compile()
    return nc


def _get_nc():
    if "nc" not in _CACHED:
        _CACHED["nc"] = _build()
    return _CACHED["nc"]


def _make_in_maps(x, Wq, bq, Wk, bk, Wv, bv, Wo):
    in_maps = []
    xts = [np.ascontiguousarray(x[b].T).astype(NPBF16) for b in range(2)]
    Wq_b = Wq.astype(NPBF16)
    Wk_b = Wk.astype(NPBF16)
    Wv_b = Wv.astype(NPBF16)
    Wo_b = Wo.astype(NPBF16)
    for core in range(8):
        b, g = divmod(core, 4)
        in_maps.append({
            "xt": xts[b],
            "wq": np.ascontiguousarray(Wq_b[:, g * GD:(g + 1) * GD]),
            "wk": np.ascontiguousarray(Wk_b[:, g * HD:(g + 1) * HD]),
            "wv": np.ascontiguousarray(Wv_b[:, g * HD:(g + 1) * HD]),
            "wo": np.ascontiguousarray(Wo_b[g * GD:(g + 1) * GD, :]),
            "bq": np.ascontiguousarray(
                bq[g * GD:(g + 1) * GD].reshape(R, 128).T
            ),
            "bk": bk[g * HD:(g + 1) * HD].reshape(HD, 1).copy(),
            "bv": bv[g * HD:(g + 1) * HD].reshape(HD, 1).copy(),
        })
    return in_maps


def kernel(x, Wq, bq, Wk, bk, Wv, bv, Wo, bo, _trace=False):
    x = np.asarray(x, dtype=np.float32)
    nc = _get_nc()
    in_maps = _make_in_maps(
        x,
        np.asarray(Wq, np.float32), np.asarray(bq, np.float32),
        np.asarray(Wk, np.float32), np.asarray(bk, np.float32),
        np.asarray(Wv, np.float32), np.asarray(bv, np.float32),
        np.asarray(Wo, np.float32),
    )
    res = run_bass_kernel_spmd(nc, in_maps, list(range(8)), trace=_trace)
    bo = np.asarray(bo, np.float32)
    out = np.empty((2, S, D), np.float32)
    for b in range(2):
        acc = res.results[b * 4]["out"].astype(np.float32)
        for g in range(1, 4):
            acc = acc + res.results[b * 4 + g]["out"].astype(np.float32)
        out[b] = acc + bo[None, :]
    if _trace:
        return out, res
    return out


# revision 9
# speedup vs baseline: 1.4991x; 1.4991x over previous
"""GQA attention block on 8 Trainium2 cores — all-bf16 matmul pipeline.

Sharding: data-parallel over batch B=2 x tensor-parallel over the 4 KV groups
(cores 0-3 -> batch 0 groups 0-3, cores 4-7 -> batch 1 groups 0-3).
Each core computes Q/K/V projections for its group, attention for its 4 query
heads, and a row-sharded partial of the output projection.  The host sums the
4 partials per batch (fp32) and adds the output bias.

v2 changes vs the fp32r baseline:
  - every matmul operand is bf16 (fp32 PSUM accumulation).  bf16 stationary
    weights get Fast Weight Load; the fp32r baseline measured 324 ns per
    128x128x512 matmul vs the 216 ns warm roofline, most of it the 4-byte
    weight-load path.
  - Q/K/V bias-adds moved from Scalar(ACT) to Vector (tensor_scalar_add) so
    ACT does nothing but the softmax EXPs (hard floor (N+352)/1.2 ns each).
  - softmax denominator partial sums split Vector/GpSimd, recip broadcast on
    GpSimd (partition_broadcast) instead of a PE ones-matmul + ACT copy.
  - softmax tails and the out-projection of chunk sc are spread as filler
    into the ACT-paced slots of the next chunk's score/attnV loops, keeping
    the PE stream dense.
  - OUT partials are bf16 (halves output DMA); host accumulates in fp32.
"""
import sys

sys.path.insert(0, "/opt/trn_rl_repo")

import math
from contextlib import ExitStack

import numpy as np
import ml_dtypes

import concourse.bacc as bacc
import concourse.tile as tile
import concourse.mybir as mybir
from concourse.bass_utils import run_bass_kernel_spmd
from concourse.masks import make_identity

F32 = mybir.dt.float32
BF16 = mybir.dt.bfloat16
AF = mybir.ActivationFunctionType
NPBF16 = ml_dtypes.bfloat16

D = 2048          # d_model
S = 2048          # sequence length
HD = 128          # head dim
R = 4             # q heads per kv group (on one core)
GD = R * HD       # 512: q-projection width per core
KT_TILES = S // 128   # 16 key-time tiles
KD_TILES = D // 128   # 16 contraction tiles for projections
N_SC = 4          # s-chunks of 512
SC = S // N_SC    # 512
SCALE = 1.0 / math.sqrt(HD)

_CACHED = {}


def _build():
    nc = bacc.Bacc("TRN2", target_bir_lowering=False, debug=False, num_devices=8)

    XT = nc.dram_tensor("xt", [D, S], BF16, kind="ExternalInput")
    WQ = nc.dram_tensor("wq", [D, GD], BF16, kind="ExternalInput")
    WK = nc.dram_tensor("wk", [D, HD], BF16, kind="ExternalInput")
    WV = nc.dram_tensor("wv", [D, HD], BF16, kind="ExternalInput")
    WO = nc.dram_tensor("wo", [GD, D], BF16, kind="ExternalInput")
    BQ = nc.dram_tensor("bq", [128, R], F32, kind="ExternalInput")
    BK = nc.dram_tensor("bk", [128, 1], F32, kind="ExternalInput")
    BV = nc.dram_tensor("bv", [128, 1], F32, kind="ExternalInput")
    OUT = nc.dram_tensor("out", [S, D], BF16, kind="ExternalOutput")

    with tile.TileContext(nc) as tc, ExitStack() as ctx:
        # ---- long-lived tiles ----
        lp = ctx.enter_context(tc.tile_pool(name="long", bufs=1))
        qt_sb = lp.tile([128, R, S], BF16)        # Q^T per head: [dq, h, s]
        kt_sb = lp.tile([128, S], BF16)           # K^T: [dk, t]
        vt_sb = lp.tile([128, S], BF16)           # V^T: [dv, t]
        v_sb = lp.tile([128, KT_TILES, HD], BF16) # V natural: [t_sub, t_tile, dv]
        bq_sb = lp.tile([128, R], F32)
        bk_sb = lp.tile([128, 1], F32)
        bv_sb = lp.tile([128, 1], F32)
        ones_col = lp.tile([128, 1], BF16)
        ones_row = lp.tile([1, 128], BF16)
        ident = lp.tile([128, 128], BF16)

        nc.sync.dma_start(bq_sb[:], BQ.ap())
        nc.sync.dma_start(bk_sb[:], BK.ap())
        nc.sync.dma_start(bv_sb[:], BV.ap())

        nc.gpsimd.memset(ones_col[:], 1.0)
        nc.gpsimd.memset(ones_row[:], 1.0)
        make_identity(nc, ident[:])

        # ---- phase A: projections ----
        with ExitStack() as actx:
            wp = actx.enter_context(tc.tile_pool(name="wqkv", bufs=1))
            xp = actx.enter_context(tc.tile_pool(name="xt", bufs=2))
            psa = actx.enter_context(tc.tile_pool(name="psa", bufs=4, space="PSUM"))

            wq_sb = wp.tile([128, KD_TILES, GD], BF16)
            wk_sb = wp.tile([128, KD_TILES, HD], BF16)
            wv_sb = wp.tile([128, KD_TILES, HD], BF16)
            wq_r = WQ.ap().rearrange("(ko p) n -> p ko n", p=128)
            wk_r = WK.ap().rearrange("(ko p) n -> p ko n", p=128)
            wv_r = WV.ap().rearrange("(ko p) n -> p ko n", p=128)

            for sc in range(N_SC):
                xt = xp.tile([128, KD_TILES, SC], BF16, tag="xt")
                xt_r = XT.ap()[:, sc * SC:(sc + 1) * SC].rearrange(
                    "(ko p) s -> p ko s", p=128
                )
                # interleave per-k so the k=0 matmuls can start almost
                # immediately (weights ride along with the first chunk)
                for k in range(KD_TILES):
                    nc.sync.dma_start(xt[:, k, :], xt_r[:, k, :])
                    if sc == 0:
                        nc.sync.dma_start(wq_sb[:, k, :], wq_r[:, k, :])
                        nc.sync.dma_start(wk_sb[:, k, :], wk_r[:, k, :])
                        nc.sync.dma_start(wv_sb[:, k, :], wv_r[:, k, :])
                # Q^T for the 4 heads
                for dq in range(R):
                    ps = psa.tile([128, SC], F32, tag="psa")
                    for k in range(KD_TILES):
                        nc.tensor.matmul(
                            ps[:],
                            lhsT=wq_sb[:, k, dq * 128:(dq + 1) * 128],
                            rhs=xt[:, k, :],
                            start=(k == 0),
                            stop=(k == KD_TILES - 1),
                        )
                    nc.vector.tensor_scalar_add(
                        qt_sb[:, dq, sc * SC:(sc + 1) * SC], ps[:],
                        bq_sb[:, dq:dq + 1],
                    )
                # K^T
                ps = psa.tile([128, SC], F32, tag="psa")
                for k in range(KD_TILES):
                    nc.tensor.matmul(
                        ps[:], lhsT=wk_sb[:, k, :], rhs=xt[:, k, :],
                        start=(k == 0), stop=(k == KD_TILES - 1),
                    )
                nc.vector.tensor_scalar_add(
                    kt_sb[:, sc * SC:(sc + 1) * SC], ps[:], bk_sb[:],
                )
                # V^T
                ps = psa.tile([128, SC], F32, tag="psa")
                for k in range(KD_TILES):
                    nc.tensor.matmul(
                        ps[:], lhsT=wv_sb[:, k, :], rhs=xt[:, k, :],
                        start=(k == 0), stop=(k == KD_TILES - 1),
                    )
                nc.vector.tensor_scalar_add(
                    vt_sb[:, sc * SC:(sc + 1) * SC], ps[:], bv_sb[:],
                )

            # V^T -> V natural (16 PE transposes)
            pst = actx.enter_context(tc.tile_pool(name="pst", bufs=2, space="PSUM"))
            for t in range(KT_TILES):
                pt_ps = pst.tile([128, 128], BF16, tag="pst")
                nc.tensor.transpose(
                    pt_ps[:], vt_sb[:, t * 128:(t + 1) * 128], ident[:]
                )
                nc.vector.tensor_copy(v_sb[:, t, :], pt_ps[:])

        # ---- phase B: attention + out-proj ----
        with ExitStack() as bctx:
            wop = bctx.enter_context(tc.tile_pool(name="wo", bufs=1))
            wo_sb = wop.tile([128, R, D], BF16)
            nc.sync.dma_start(wo_sb[:], WO.ap().rearrange("(h p) n -> p h n", p=128))

            pss = bctx.enter_context(tc.tile_pool(name="pss", bufs=2, space="PSUM"))
            pso = bctx.enter_context(tc.tile_pool(name="pso", bufs=2, space="PSUM"))
            psm = bctx.enter_context(tc.tile_pool(name="psm", bufs=2, space="PSUM"))
            ptp = bctx.enter_context(tc.tile_pool(name="ptp", bufs=2))
            accp = bctx.enter_context(tc.tile_pool(name="accp", bufs=2))
            otp = bctx.enter_context(tc.tile_pool(name="otp", bufs=2))
            outp = bctx.enter_context(tc.tile_pool(name="outp", bufs=4))

            # filler queues consumed inside t_loop slots.  The tail of combo
            # i is emitted at combo i+1's tg==5 slot (late enough that the
            # slow gpsimd denominator chain of combo i has finished, so the
            # PE's ps_d matmul never stalls the in-order PE stream).
            # out-proj groups of chunk sc are gated until tail(sc,3) emitted.
            tail_q = []
            op_q = []

            def emit_filler(tg):
                if tg == 5 and tail_q:
                    tail_q.pop(0)()
                elif tg != 5 and op_q:
                    op_q.pop(0)()

            def t_loop(sc, h):
                """scores -> exp -> attnV accumulation + split denom sums."""
                pt = ptp.tile([128, KT_TILES, SC], BF16, tag="pt", name="pt")
                accD = accp.tile([128, SC], BF16, tag="accD", name="accD")
                accG = accp.tile([128, SC], BF16, tag="accG", name="accG")
                ps_o = pso.tile([128, SC], F32, tag="pso", name="ps_o")
                q_ap = qt_sb[:, h, sc * SC:(sc + 1) * SC]

                def scores(tg):
                    ps_s = pss.tile([128, 2, SC], F32, tag="pss", name="ps_s")
                    for i in range(2):
                        t = tg * 2 + i
                        nc.tensor.matmul(
                            ps_s[:, i, :],
                            lhsT=kt_sb[:, t * 128:(t + 1) * 128],
                            rhs=q_ap,
                            start=True, stop=True,
                        )
                    return ps_s

                cur = scores(0)
                for tg in range(KT_TILES // 2):
                    nxt = scores(tg + 1) if tg < KT_TILES // 2 - 1 else None
                    nc.scalar.activation(
                        pt[:, 2 * tg:2 * tg + 2, :], cur[:], AF.Exp, scale=SCALE
                    )
                    for i in range(2):
                        t = tg * 2 + i
                        nc.tensor.matmul(
                            ps_o[:],
                            lhsT=v_sb[:, t, :],
                            rhs=pt[:, t, :],
                            start=(t == 0),
                            stop=(t == KT_TILES - 1),
                            skip_group_check=True,
                        )
                    # denominator partials: DVE owns tiles 0..9, gpsimd 10..15
                    # (gpsimd runs ONLY tensor_add all kernel long — mixing op
                    # kinds forces a DSP LIBRARY_RELOAD each switch).  First op
                    # of each chain adds two pt tiles to avoid a copy.
                    if tg < 5:
                        if tg == 0:
                            nc.vector.tensor_add(accD[:], pt[:, 0, :], pt[:, 1, :])
                        else:
                            for i in range(2):
                                t = tg * 2 + i
                                nc.vector.tensor_add(accD[:], accD[:], pt[:, t, :])
                    else:
                        if tg == 5:
                            nc.gpsimd.tensor_add(accG[:], pt[:, 10, :], pt[:, 11, :])
                        else:
                            for i in range(2):
                                t = tg * 2 + i
                                nc.gpsimd.tensor_add(accG[:], accG[:], pt[:, t, :])
                    emit_filler(tg)
                    cur = nxt
                return ps_o, accD, accG

            def tail(sc, h, ot_sb, ps_o, accD, accG):
                """denominator -> reciprocal -> broadcast -> normalize."""
                acc_r = accp.tile([128, SC], BF16, tag="accr", name="acc_r")
                nc.vector.tensor_add(acc_r[:], accD[:], accG[:])
                ps_d = psm.tile([1, SC], F32, tag="psm", name="ps_d")
                nc.tensor.matmul(
                    ps_d[:], lhsT=ones_col[:], rhs=acc_r[:], start=True, stop=True
                )
                recip = accp.tile([1, SC], F32, tag="recip", name="recip")
                nc.vector.reciprocal_approx_fast(recip[:], ps_d[:])
                recip_b = accp.tile([1, SC], BF16, tag="recipb", name="recip_b")
                nc.vector.tensor_copy(recip_b[:], recip[:])
                ps_b = psm.tile([128, SC], F32, tag="psm", name="ps_b")
                nc.tensor.matmul(
                    ps_b[:], lhsT=ones_row[:], rhs=recip_b[:], start=True, stop=True
                )
                bc = accp.tile([128, SC], F32, tag="bc", name="bc")
                nc.vector.tensor_copy(bc[:], ps_b[:])
                nc.vector.tensor_mul(ot_sb[:, h, :], ps_o[:], bc[:])
                if h == R - 1:
                    op_q.extend(op_pending.pop(0))

            op_pending = []  # per-sc out-proj group lists, released by tail(sc,3)

            def queue_out_proj(sc, ot_sb):
                groups = []
                for st in range(SC // 128):
                    for oc in range(D // 512):
                        def go(st=st, oc=oc):
                            ps_f = psm.tile([128, 512], F32, tag="psm", name="ps_f")
                            for dv in range(R):
                                nc.tensor.matmul(
                                    ps_f[:],
                                    lhsT=ot_sb[:, dv, st * 128:(st + 1) * 128],
                                    rhs=wo_sb[:, dv, oc * 512:(oc + 1) * 512],
                                    start=(dv == 0),
                                    stop=(dv == R - 1),
                                    skip_group_check=True,
                                )
                            o_t = outp.tile([128, 512], BF16, tag="out", name="o_t")
                            nc.vector.tensor_copy(o_t[:], ps_f[:])
                            nc.sync.dma_start(
                                OUT.ap()[
                                    sc * SC + st * 128: sc * SC + (st + 1) * 128,
                                    oc * 512:(oc + 1) * 512,
                                ],
                                o_t[:],
                            )
                        groups.append(go)
                op_pending.append(groups)

            for sc in range(N_SC):
                ot_sb = otp.tile([128, R, SC], BF16, tag="ot", name="ot_sb")
                for h in range(R):
                    ps_o, accD, accG = t_loop(sc, h)
                    tail_q.append(
                        lambda sc=sc, h=h, ot_sb=ot_sb, ps_o=ps_o, accD=accD,
                        accG=accG: tail(sc, h, ot_sb, ps_o, accD, accG)
                    )
                queue_out_proj(sc, ot_sb)
            while tail_q:
                tail_q.pop(0)()
            while op_q:
                op_q.pop(0)()

    nc.compile()
    return nc


def _get_nc():
    if "nc" not in _CACHED:
        _CACHED["nc"] = _build()
    return _CACHED["nc"]


def _make_in_maps(x, Wq, bq, Wk, bk, Wv, bv, Wo):
    in_maps = []
    xts = [np.ascontiguousarray(x[b].T).astype(NPBF16) for b in range(2)]
    Wq_b = Wq.astype(NPBF16)
    Wk_b = Wk.astype(NPBF16)
    Wv_b = Wv.astype(NPBF16)
    Wo_b = Wo.astype(NPBF16)
    for core in range(8):
        b, g = divmod(core, 4)
        in_maps.append({
            "xt": xts[b],
            "wq": np.ascontiguousarray(Wq_b[:, g * GD:(g + 1) * GD]),
            "wk": np.ascontiguousarray(Wk_b[:, g * HD:(g + 1) * HD]),
            "wv": np.ascontiguousarray(Wv_b[:, g * HD:(g + 1) * HD]),
            "wo": np.ascontiguousarray(Wo_b[g * GD:(g + 1) * GD, :]),
            "bq": np.ascontiguousarray(
                bq[g * GD:(g + 1) * GD].reshape(R, 128).T
            ),
            "bk": bk[g * HD:(g + 1) * HD].reshape(HD, 1).copy(),
            "bv": bv[g * HD:(g + 1) * HD].reshape(HD, 1).copy(),
        })
    return in_maps


def kernel(x, Wq, bq, Wk, bk, Wv, bv, Wo, bo, _trace=False):
    x = np.asarray(x, dtype=np.float32)
    nc = _get_nc()
    in_maps = _make_in_maps(
        x,
        np.asarray(Wq, np.float32), np.asarray(bq, np.float32),
        np.asarray(Wk, np.float32), np.asarray(bk, np.float32),
        np.asarray(Wv, np.float32), np.asarray(bv, np.float32),
        np.asarray(Wo, np.float32),
    )
    res = run_bass_kernel_spmd(nc, in_maps, list(range(8)), trace=_trace)
    bo = np.asarray(bo, np.float32)
    out = np.empty((2, S, D), np.float32)
    for b in range(2):
        acc = res.results[b * 4]["out"].astype(np.float32)
        for g in range(1, 4):
            acc = acc + res.results[b * 4 + g]["out"].astype(np.float32)
        out[b] = acc + bo[None, :]
    if _trace:
        return out, res
    return out


# revision 13
# speedup vs baseline: 1.5216x; 1.0150x over previous
"""GQA attention block on 8 Trainium2 cores — all-bf16 matmul pipeline.

Sharding: data-parallel over batch B=2 x tensor-parallel over the 4 KV groups
(cores 0-3 -> batch 0 groups 0-3, cores 4-7 -> batch 1 groups 0-3).
Each core computes Q/K/V projections for its group, attention for its 4 query
heads, and a row-sharded partial of the output projection.  The host sums the
4 partials per batch (fp32) and adds the output bias.

v2 changes vs the fp32r baseline:
  - every matmul operand is bf16 (fp32 PSUM accumulation).  bf16 stationary
    weights get Fast Weight Load; the fp32r baseline measured 324 ns per
    128x128x512 matmul vs the 216 ns warm roofline, most of it the 4-byte
    weight-load path.
  - Q/K/V bias-adds moved from Scalar(ACT) to Vector (tensor_scalar_add) so
    ACT does nothing but the softmax EXPs (hard floor (N+352)/1.2 ns each).
  - softmax denominator partial sums split Vector/GpSimd, recip broadcast on
    GpSimd (partition_broadcast) instead of a PE ones-matmul + ACT copy.
  - softmax tails and the out-projection of chunk sc are spread as filler
    into the ACT-paced slots of the next chunk's score/attnV loops, keeping
    the PE stream dense.
  - OUT partials are bf16 (halves output DMA); host accumulates in fp32.
"""
import sys

sys.path.insert(0, "/opt/trn_rl_repo")

import math
from contextlib import ExitStack

import numpy as np
import ml_dtypes

import concourse.bacc as bacc
import concourse.tile as tile
import concourse.mybir as mybir
from concourse.bass_utils import run_bass_kernel_spmd
from concourse.masks import make_identity

F32 = mybir.dt.float32
BF16 = mybir.dt.bfloat16
AF = mybir.ActivationFunctionType
NPBF16 = ml_dtypes.bfloat16

D = 2048          # d_model
S = 2048          # sequence length
HD = 128          # head dim
R = 4             # q heads per kv group (on one core)
GD = R * HD       # 512: q-projection width per core
KT_TILES = S // 128   # 16 key-time tiles
KD_TILES = D // 128   # 16 contraction tiles for projections
N_SC = 4          # s-chunks of 512
SC = S // N_SC    # 512
SCALE = 1.0 / math.sqrt(HD)

_CACHED = {}


def _build():
    nc = bacc.Bacc("TRN2", target_bir_lowering=False, debug=False, num_devices=8)

    XT = nc.dram_tensor("xt", [D, S], BF16, kind="ExternalInput")
    WQ = nc.dram_tensor("wq", [D, GD], BF16, kind="ExternalInput")
    WK = nc.dram_tensor("wk", [D, HD], BF16, kind="ExternalInput")
    WV = nc.dram_tensor("wv", [D, HD], BF16, kind="ExternalInput")
    WO = nc.dram_tensor("wo", [GD, D], BF16, kind="ExternalInput")
    BQ = nc.dram_tensor("bq", [128, R], F32, kind="ExternalInput")
    BK = nc.dram_tensor("bk", [128, 1], F32, kind="ExternalInput")
    BV = nc.dram_tensor("bv", [128, 1], F32, kind="ExternalInput")
    OUT = nc.dram_tensor("out", [S, D], BF16, kind="ExternalOutput")

    with tile.TileContext(nc) as tc, ExitStack() as ctx:
        # ---- long-lived tiles ----
        lp = ctx.enter_context(tc.tile_pool(name="long", bufs=1))
        qt_sb = lp.tile([128, R, S], BF16)        # Q^T per head: [dq, h, s]
        kt_sb = lp.tile([128, S], BF16)           # K^T: [dk, t]
        vt_sb = lp.tile([128, S], BF16)           # V^T: [dv, t]
        v_sb = lp.tile([128, KT_TILES, HD], BF16) # V natural: [t_sub, t_tile, dv]
        bq_sb = lp.tile([128, R], F32)
        bk_sb = lp.tile([128, 1], F32)
        bv_sb = lp.tile([128, 1], F32)
        ones_col = lp.tile([128, 1], BF16)
        ones_row = lp.tile([1, 128], BF16)
        ident = lp.tile([128, 128], BF16)

        nc.sync.dma_start(bq_sb[:], BQ.ap())
        nc.sync.dma_start(bk_sb[:], BK.ap())
        nc.sync.dma_start(bv_sb[:], BV.ap())

        nc.gpsimd.memset(ones_col[:], 1.0)
        nc.gpsimd.memset(ones_row[:], 1.0)
        make_identity(nc, ident[:])

        # ---- phase A: projections ----
        with ExitStack() as actx:
            wp = actx.enter_context(tc.tile_pool(name="wqkv", bufs=1))
            xp = actx.enter_context(tc.tile_pool(name="xt", bufs=2))
            psa = actx.enter_context(tc.tile_pool(name="psa", bufs=4, space="PSUM"))

            wq_sb = wp.tile([128, KD_TILES, GD], BF16)
            wk_sb = wp.tile([128, KD_TILES, HD], BF16)
            wv_sb = wp.tile([128, KD_TILES, HD], BF16)
            wq_r = WQ.ap().rearrange("(ko p) n -> p ko n", p=128)
            wk_r = WK.ap().rearrange("(ko p) n -> p ko n", p=128)
            wv_r = WV.ap().rearrange("(ko p) n -> p ko n", p=128)

            for sc in range(N_SC):
                xt = xp.tile([128, KD_TILES, SC], BF16, tag="xt")
                xt_r = XT.ap()[:, sc * SC:(sc + 1) * SC].rearrange(
                    "(ko p) s -> p ko s", p=128
                )
                # interleave per-k so the k=0 matmuls can start almost
                # immediately (weights ride along with the first chunk);
                # wk/wv/ arrive while the Q matmuls run.
                for k in range(KD_TILES):
                    nc.sync.dma_start(xt[:, k, :], xt_r[:, k, :])
                    if sc == 0:
                        nc.sync.dma_start(wq_sb[:, k, :], wq_r[:, k, :])
                if sc == 0:
                    for k in range(KD_TILES):
                        nc.sync.dma_start(wk_sb[:, k, :], wk_r[:, k, :])
                        nc.sync.dma_start(wv_sb[:, k, :], wv_r[:, k, :])
                # Q^T for the 4 heads, k-outer so each weight/x chunk is
                # reused by 4 matmuls as soon as it lands (keeps the start
                # of phase A PE-paced, not DMA-paced)
                ps_q = [psa.tile([128, SC], F32, tag=f"psa{dq}", bufs=1,
                                 name=f"ps_q{dq}") for dq in range(R)]
                for k in range(KD_TILES):
                    for dq in range(R):
                        nc.tensor.matmul(
                            ps_q[dq][:],
                            lhsT=wq_sb[:, k, dq * 128:(dq + 1) * 128],
                            rhs=xt[:, k, :],
                            start=(k == 0),
                            stop=(k == KD_TILES - 1),
                            skip_group_check=True,
                        )
                for dq in range(R):
                    nc.vector.tensor_scalar_add(
                        qt_sb[:, dq, sc * SC:(sc + 1) * SC], ps_q[dq][:],
                        bq_sb[:, dq:dq + 1],
                    )
                # K^T and V^T, k-outer
                ps_k = psa.tile([128, SC], F32, tag="psa0", bufs=1, name="ps_k")
                ps_v = psa.tile([128, SC], F32, tag="psa1", bufs=1, name="ps_v")
                for k in range(KD_TILES):
                    nc.tensor.matmul(
                        ps_k[:], lhsT=wk_sb[:, k, :], rhs=xt[:, k, :],
                        start=(k == 0), stop=(k == KD_TILES - 1),
                        skip_group_check=True,
                    )
                    nc.tensor.matmul(
                        ps_v[:], lhsT=wv_sb[:, k, :], rhs=xt[:, k, :],
                        start=(k == 0), stop=(k == KD_TILES - 1),
                        skip_group_check=True,
                    )
                nc.vector.tensor_scalar_add(
                    kt_sb[:, sc * SC:(sc + 1) * SC], ps_k[:], bk_sb[:],
                )
                nc.vector.tensor_scalar_add(
                    vt_sb[:, sc * SC:(sc + 1) * SC], ps_v[:], bv_sb[:],
                )

            # V^T -> V natural (16 PE transposes)
            pst = actx.enter_context(tc.tile_pool(name="pst", bufs=2, space="PSUM"))
            for t in range(KT_TILES):
                pt_ps = pst.tile([128, 128], BF16, tag="pst")
                nc.tensor.transpose(
                    pt_ps[:], vt_sb[:, t * 128:(t + 1) * 128], ident[:]
                )
                nc.vector.tensor_copy(v_sb[:, t, :], pt_ps[:])

        # ---- phase B: attention + out-proj ----
        with ExitStack() as bctx:
            wop = bctx.enter_context(tc.tile_pool(name="wo", bufs=1))
            wo_sb = wop.tile([128, R, D], BF16)
            nc.sync.dma_start(wo_sb[:], WO.ap().rearrange("(h p) n -> p h n", p=128))

            pss = bctx.enter_context(tc.tile_pool(name="pss", bufs=2, space="PSUM"))
            pso = bctx.enter_context(tc.tile_pool(name="pso", bufs=2, space="PSUM"))
            psm = bctx.enter_context(tc.tile_pool(name="psm", bufs=2, space="PSUM"))
            ptp = bctx.enter_context(tc.tile_pool(name="ptp", bufs=2))
            accp = bctx.enter_context(tc.tile_pool(name="accp", bufs=2))
            otp = bctx.enter_context(tc.tile_pool(name="otp", bufs=2))
            outp = bctx.enter_context(tc.tile_pool(name="outp", bufs=4))

            # filler queues consumed inside t_loop slots.  The tail of combo
            # i is emitted at combo i+1's tg==5 slot (late enough that the
            # slow gpsimd denominator chain of combo i has finished, so the
            # PE's ps_d matmul never stalls the in-order PE stream).
            # out-proj groups of chunk sc are gated until tail(sc,3) emitted.
            tail_q = []
            op_q = []

            def emit_filler(tg):
                if tg == 5 and tail_q:
                    tail_q.pop(0)()
                elif tg != 5 and op_q:
                    op_q.pop(0)()

            def t_loop(sc, h):
                """scores -> exp -> attnV accumulation + split denom sums."""
                pt = ptp.tile([128, KT_TILES, SC], BF16, tag="pt", name="pt")
                accD = accp.tile([128, SC], BF16, tag="accD", name="accD")
                accG = accp.tile([128, SC], BF16, tag="accG", name="accG")
                ps_o = pso.tile([128, SC], F32, tag="pso", name="ps_o")
                q_ap = qt_sb[:, h, sc * SC:(sc + 1) * SC]

                def scores(tg):
                    ps_s = pss.tile([128, 2, SC], F32, tag="pss", name="ps_s")
                    for i in range(2):
                        t = tg * 2 + i
                        nc.tensor.matmul(
                            ps_s[:, i, :],
                            lhsT=kt_sb[:, t * 128:(t + 1) * 128],
                            rhs=q_ap,
                            start=True, stop=True,
                        )
                    return ps_s

                cur = scores(0)
                for tg in range(KT_TILES // 2):
                    nxt = scores(tg + 1) if tg < KT_TILES // 2 - 1 else None
                    nc.scalar.activation(
                        pt[:, 2 * tg:2 * tg + 2, :], cur[:], AF.Exp, scale=SCALE
                    )
                    for i in range(2):
                        t = tg * 2 + i
                        nc.tensor.matmul(
                            ps_o[:],
                            lhsT=v_sb[:, t, :],
                            rhs=pt[:, t, :],
                            start=(t == 0),
                            stop=(t == KT_TILES - 1),
                            skip_group_check=True,
                        )
                    # denominator partials: gpsimd owns the EARLY tiles 0..7
                    # (slow ~1.2us/add, but its chain becomes eligible from
                    # exp(0) so it has the whole combo to finish); DVE owns
                    # the late tiles 8..15 (fast, finishes right after exp7).
                    # gpsimd runs ONLY tensor_add all kernel long — mixing op
                    # kinds forces a DSP LIBRARY_RELOAD each switch.  First op
                    # of each chain adds two pt tiles to avoid a copy.
                    if tg < 4:
                        if tg == 0:
                            nc.gpsimd.tensor_add(accG[:], pt[:, 0, :], pt[:, 1, :])
                        else:
                            for i in range(2):
                                t = tg * 2 + i
                                nc.gpsimd.tensor_add(accG[:], accG[:], pt[:, t, :])
                    else:
                        if tg == 4:
                            nc.vector.tensor_add(accD[:], pt[:, 8, :], pt[:, 9, :])
                        else:
                            for i in range(2):
                                t = tg * 2 + i
                                nc.vector.tensor_add(accD[:], accD[:], pt[:, t, :])
                    emit_filler(tg)
                    cur = nxt
                return ps_o, accD, accG

            def tail(sc, h, ot_sb, ps_o, accD, accG):
                """denominator -> reciprocal -> broadcast -> normalize."""
                acc_r = accp.tile([128, SC], BF16, tag="accr", name="acc_r")
                nc.vector.tensor_add(acc_r[:], accD[:], accG[:])
                ps_d = psm.tile([1, SC], F32, tag="psm", name="ps_d")
                nc.tensor.matmul(
                    ps_d[:], lhsT=ones_col[:], rhs=acc_r[:], start=True, stop=True
                )
                recip = accp.tile([1, SC], F32, tag="recip", name="recip")
                nc.vector.reciprocal_approx_fast(recip[:], ps_d[:])
                recip_b = accp.tile([1, SC], BF16, tag="recipb", name="recip_b")
                nc.vector.tensor_copy(recip_b[:], recip[:])
                ps_b = psm.tile([128, SC], F32, tag="psm", name="ps_b")
                nc.tensor.matmul(
                    ps_b[:], lhsT=ones_row[:], rhs=recip_b[:], start=True, stop=True
                )
                bc = accp.tile([128, SC], F32, tag="bc", name="bc")
                nc.vector.tensor_copy(bc[:], ps_b[:])
                nc.vector.tensor_mul(ot_sb[:, h, :], ps_o[:], bc[:])
                if h == R - 1:
                    op_q.extend(op_pending.pop(0))

            op_pending = []  # per-sc out-proj group lists, released by tail(sc,3)

            def queue_out_proj(sc, ot_sb):
                groups = []
                for st in range(SC // 128):
                    for oc in range(D // 512):
                        def go(st=st, oc=oc):
                            ps_f = psm.tile([128, 512], F32, tag="psm", name="ps_f")
                            for dv in range(R):
                                nc.tensor.matmul(
                                    ps_f[:],
                                    lhsT=ot_sb[:, dv, st * 128:(st + 1) * 128],
                                    rhs=wo_sb[:, dv, oc * 512:(oc + 1) * 512],
                                    start=(dv == 0),
                                    stop=(dv == R - 1),
                                    skip_group_check=True,
                                )
                            o_t = outp.tile([128, 512], BF16, tag="out", name="o_t")
                            nc.vector.tensor_copy(o_t[:], ps_f[:])
                            nc.sync.dma_start(
                                OUT.ap()[
                                    sc * SC + st * 128: sc * SC + (st + 1) * 128,
                                    oc * 512:(oc + 1) * 512,
                                ],
                                o_t[:],
                            )
                        groups.append(go)
                op_pending.append(groups)

            for sc in range(N_SC):
                ot_sb = otp.tile([128, R, SC], BF16, tag="ot", name="ot_sb")
                for h in range(R):
                    ps_o, accD, accG = t_loop(sc, h)
                    tail_q.append(
                        lambda sc=sc, h=h, ot_sb=ot_sb, ps_o=ps_o, accD=accD,
                        accG=accG: tail(sc, h, ot_sb, ps_o, accD, accG)
                    )
                queue_out_proj(sc, ot_sb)
            while tail_q:
                tail_q.pop(0)()
            while op_q:
                op_q.pop(0)()

    nc.compile()
    return nc


def _get_nc():
    if "nc" not in _CACHED:
        _CACHED["nc"] = _build()
    return _CACHED["nc"]


def _make_in_maps(x, Wq, bq, Wk, bk, Wv, bv, Wo):
    in_maps = []
    xts = [np.ascontiguousarray(x[b].T).astype(NPBF16) for b in range(2)]
    Wq_b = Wq.astype(NPBF16)
    Wk_b = Wk.astype(NPBF16)
    Wv_b = Wv.astype(NPBF16)
    Wo_b = Wo.astype(NPBF16)
    for core in range(8):
        b, g = divmod(core, 4)
        in_maps.append({
            "xt": xts[b],
            "wq": np.ascontiguousarray(Wq_b[:, g * GD:(g + 1) * GD]),
            "wk": np.ascontiguousarray(Wk_b[:, g * HD:(g + 1) * HD]),
            "wv": np.ascontiguousarray(Wv_b[:, g * HD:(g + 1) * HD]),
            "wo": np.ascontiguousarray(Wo_b[g * GD:(g + 1) * GD, :]),
            "bq": np.ascontiguousarray(
                bq[g * GD:(g + 1) * GD].reshape(R, 128).T
            ),
            "bk": bk[g * HD:(g + 1) * HD].reshape(HD, 1).copy(),
            "bv": bv[g * HD:(g + 1) * HD].reshape(HD, 1).copy(),
        })
    return in_maps


def kernel(x, Wq, bq, Wk, bk, Wv, bv, Wo, bo, _trace=False):
    x = np.asarray(x, dtype=np.float32)
    nc = _get_nc()
    in_maps = _make_in_maps(
        x,
        np.asarray(Wq, np.float32), np.asarray(bq, np.float32),
        np.asarray(Wk, np.float32), np.asarray(bk, np.float32),
        np.asarray(Wv, np.float32), np.asarray(bv, np.float32),
        np.asarray(Wo, np.float32),
    )
    res = run_bass_kernel_spmd(nc, in_maps, list(range(8)), trace=_trace)
    bo = np.asarray(bo, np.float32)
    out = np.empty((2, S, D), np.float32)
    for b in range(2):
        acc = res.results[b * 4]["out"].astype(np.float32)
        for g in range(1, 4):
            acc = acc + res.results[b * 4 + g]["out"].astype(np.float32)
        out[b] = acc + bo[None, :]
    if _trace:
        return out, res
    return out


# revision 20
# speedup vs baseline: 1.5637x; 1.0277x over previous
"""GQA attention block on 8 Trainium2 cores — all-bf16 matmul pipeline.

Sharding: data-parallel over batch B=2 x tensor-parallel over the 4 KV groups
(cores 0-3 -> batch 0 groups 0-3, cores 4-7 -> batch 1 groups 0-3).
Each core computes Q/K/V projections for its group, attention for its 4 query
heads, and a row-sharded partial of the output projection.  The host sums the
4 partials per batch (fp32) and adds the output bias.

v2 changes vs the fp32r baseline:
  - every matmul operand is bf16 (fp32 PSUM accumulation).  bf16 stationary
    weights get Fast Weight Load; the fp32r baseline measured 324 ns per
    128x128x512 matmul vs the 216 ns warm roofline, most of it the 4-byte
    weight-load path.
  - Q/K/V bias-adds moved from Scalar(ACT) to Vector (tensor_scalar_add) so
    ACT does nothing but the softmax EXPs (hard floor (N+352)/1.2 ns each).
  - softmax denominator partial sums split Vector/GpSimd, recip broadcast on
    GpSimd (partition_broadcast) instead of a PE ones-matmul + ACT copy.
  - softmax tails and the out-projection of chunk sc are spread as filler
    into the ACT-paced slots of the next chunk's score/attnV loops, keeping
    the PE stream dense.
  - OUT partials are bf16 (halves output DMA); host accumulates in fp32.
"""
import sys

sys.path.insert(0, "/opt/trn_rl_repo")

import math
from contextlib import ExitStack

import numpy as np
import ml_dtypes

import concourse.bacc as bacc
import concourse.tile as tile
import concourse.mybir as mybir
from concourse.bass_utils import run_bass_kernel_spmd
from concourse.masks import make_identity

F32 = mybir.dt.float32
BF16 = mybir.dt.bfloat16
AF = mybir.ActivationFunctionType
NPBF16 = ml_dtypes.bfloat16

D = 2048          # d_model
S = 2048          # sequence length
HD = 128          # head dim
R = 4             # q heads per kv group (on one core)
GD = R * HD       # 512: q-projection width per core
KT_TILES = S // 128   # 16 key-time tiles
KD_TILES = D // 128   # 16 contraction tiles for projections
N_SC = 4          # s-chunks of 512
SC = S // N_SC    # 512
SCALE = 1.0 / math.sqrt(HD)

_CACHED = {}


def _build():
    nc = bacc.Bacc("TRN2", target_bir_lowering=False, debug=False, num_devices=8)

    XT = nc.dram_tensor("xt", [D, S], BF16, kind="ExternalInput")
    WQ = nc.dram_tensor("wq", [D, GD], BF16, kind="ExternalInput")
    WK = nc.dram_tensor("wk", [D, HD], BF16, kind="ExternalInput")
    WV = nc.dram_tensor("wv", [D, HD], BF16, kind="ExternalInput")
    WO = nc.dram_tensor("wo", [GD, D], BF16, kind="ExternalInput")
    BQ = nc.dram_tensor("bq", [128, R], F32, kind="ExternalInput")
    BK = nc.dram_tensor("bk", [128, 1], F32, kind="ExternalInput")
    BV = nc.dram_tensor("bv", [128, 1], F32, kind="ExternalInput")
    OUT = nc.dram_tensor("out", [S, D], BF16, kind="ExternalOutput")

    with tile.TileContext(nc) as tc, ExitStack() as ctx:
        # ---- long-lived tiles ----
        lp = ctx.enter_context(tc.tile_pool(name="long", bufs=1))
        qt_sb = lp.tile([128, R, S], BF16)        # Q^T per head: [dq, h, s]
        kt_sb = lp.tile([128, S], BF16)           # K^T: [dk, t]
        vt_sb = lp.tile([128, S], BF16)           # V^T: [dv, t]
        v_sb = lp.tile([128, KT_TILES, HD], BF16) # V natural: [t_sub, t_tile, dv]
        bq_sb = lp.tile([128, R], F32)
        bk_sb = lp.tile([128, 1], F32)
        bv_sb = lp.tile([128, 1], F32)
        ones_sq = lp.tile([128, 128], BF16)
        ident = lp.tile([128, 128], BF16)

        nc.sync.dma_start(bq_sb[:], BQ.ap())
        nc.sync.dma_start(bk_sb[:], BK.ap())
        nc.sync.dma_start(bv_sb[:], BV.ap())

        nc.gpsimd.memset(ones_sq[:], 1.0)
        make_identity(nc, ident[:])

        # ---- phase A: projections ----
        with ExitStack() as actx:
            wp = actx.enter_context(tc.tile_pool(name="wqkv", bufs=1))
            xp = actx.enter_context(tc.tile_pool(name="xt", bufs=2))
            psa = actx.enter_context(tc.tile_pool(name="psa", bufs=4, space="PSUM"))

            wq_sb = wp.tile([128, KD_TILES, GD], BF16)
            wk_sb = wp.tile([128, KD_TILES, HD], BF16)
            wv_sb = wp.tile([128, KD_TILES, HD], BF16)
            wq_r = WQ.ap().rearrange("(ko p) n -> p ko n", p=128)
            wk_r = WK.ap().rearrange("(ko p) n -> p ko n", p=128)
            wv_r = WV.ap().rearrange("(ko p) n -> p ko n", p=128)

            for sc in range(N_SC):
                xt = xp.tile([128, KD_TILES, SC], BF16, tag="xt")
                xt_r = XT.ap()[:, sc * SC:(sc + 1) * SC].rearrange(
                    "(ko p) s -> p ko s", p=128
                )
                # interleave per-k so the k=0 matmuls can start almost
                # immediately (weights ride along with the first chunk);
                # wk/wv/ arrive while the Q matmuls run.
                for k in range(KD_TILES):
                    nc.sync.dma_start(xt[:, k, :], xt_r[:, k, :])
                    if sc == 0:
                        nc.sync.dma_start(wq_sb[:, k, :], wq_r[:, k, :])
                if sc == 0:
                    for k in range(KD_TILES):
                        nc.sync.dma_start(wk_sb[:, k, :], wk_r[:, k, :])
                        nc.sync.dma_start(wv_sb[:, k, :], wv_r[:, k, :])
                # Q^T for the 4 heads, k-outer so each weight/x chunk is
                # reused by 4 matmuls as soon as it lands (keeps the start
                # of phase A PE-paced, not DMA-paced)
                # psa0/psa1 are reused by K/V right after the Q heads, so they
                # get 2 buffers (frees the PE from waiting on the DVE bias
                # chain); psa2/psa3 single.  4*1 + 2*... = 6 banks + pst 2 = 8.
                ps_q = [psa.tile([128, SC], F32, tag=f"psa{dq}",
                                 bufs=(2 if dq < 2 else 1),
                                 name=f"ps_q{dq}") for dq in range(R)]
                for k in range(KD_TILES):
                    for dq in range(R):
                        nc.tensor.matmul(
                            ps_q[dq][:],
                            lhsT=wq_sb[:, k, dq * 128:(dq + 1) * 128],
                            rhs=xt[:, k, :],
                            start=(k == 0),
                            stop=(k == KD_TILES - 1),
                            skip_group_check=True,
                        )
                for dq in range(R):
                    nc.vector.tensor_scalar_add(
                        qt_sb[:, dq, sc * SC:(sc + 1) * SC], ps_q[dq][:],
                        bq_sb[:, dq:dq + 1],
                    )
                # K^T and V^T, k-outer
                ps_k = psa.tile([128, SC], F32, tag="psa0", bufs=2, name="ps_k")
                ps_v = psa.tile([128, SC], F32, tag="psa1", bufs=2, name="ps_v")
                for k in range(KD_TILES):
                    nc.tensor.matmul(
                        ps_k[:], lhsT=wk_sb[:, k, :], rhs=xt[:, k, :],
                        start=(k == 0), stop=(k == KD_TILES - 1),
                        skip_group_check=True,
                    )
                    nc.tensor.matmul(
                        ps_v[:], lhsT=wv_sb[:, k, :], rhs=xt[:, k, :],
                        start=(k == 0), stop=(k == KD_TILES - 1),
                        skip_group_check=True,
                    )
                nc.vector.tensor_scalar_add(
                    kt_sb[:, sc * SC:(sc + 1) * SC], ps_k[:], bk_sb[:],
                )
                nc.vector.tensor_scalar_add(
                    vt_sb[:, sc * SC:(sc + 1) * SC], ps_v[:], bv_sb[:],
                )

            # V^T -> V natural (16 PE transposes)
            pst = actx.enter_context(tc.tile_pool(name="pst", bufs=2, space="PSUM"))
            for t in range(KT_TILES):
                pt_ps = pst.tile([128, 128], BF16, tag="pst")
                nc.tensor.transpose(
                    pt_ps[:], vt_sb[:, t * 128:(t + 1) * 128], ident[:]
                )
                nc.vector.tensor_copy(v_sb[:, t, :], pt_ps[:])

        # ---- phase B: attention + out-proj ----
        with ExitStack() as bctx:
            wop = bctx.enter_context(tc.tile_pool(name="wo", bufs=1))
            wo_sb = wop.tile([128, R, D], BF16)
            nc.sync.dma_start(wo_sb[:], WO.ap().rearrange("(h p) n -> p h n", p=128))

            pss = bctx.enter_context(tc.tile_pool(name="pss", bufs=2, space="PSUM"))
            pso = bctx.enter_context(tc.tile_pool(name="pso", bufs=2, space="PSUM"))
            psm = bctx.enter_context(tc.tile_pool(name="psm", bufs=2, space="PSUM"))
            ptp = bctx.enter_context(tc.tile_pool(name="ptp", bufs=2))
            accp = bctx.enter_context(tc.tile_pool(name="accp", bufs=2))
            otp = bctx.enter_context(tc.tile_pool(name="otp", bufs=2))
            outp = bctx.enter_context(tc.tile_pool(name="outp", bufs=4))

            # filler queues consumed inside t_loop slots.  The tail of combo
            # i is emitted at combo i+1's tg==5 slot (late enough that the
            # slow gpsimd denominator chain of combo i has finished, so the
            # PE's ps_d matmul never stalls the in-order PE stream).
            # out-proj groups of chunk sc are gated until tail(sc,3) emitted.
            tail_q = []
            op_q = []

            def emit_filler(tg):
                if tg == 5 and tail_q:
                    tail_q.pop(0)()
                elif tg != 5 and op_q:
                    op_q.pop(0)()

            def t_loop(sc, h, pre, nxt_combo):
                """scores -> exp -> attnV accumulation + split denom sums.
                `pre` is this combo's first score pair (pre-emitted by the
                previous combo so the PE never drains at a combo boundary);
                returns the next combo's pre-emitted pair."""
                pt = ptp.tile([128, KT_TILES, SC], BF16, tag="pt", name="pt")
                accD = accp.tile([128, SC], BF16, tag="accD", name="accD")
                accG = accp.tile([128, SC], BF16, tag="accG", name="accG")
                ps_o = pso.tile([128, SC], F32, tag="pso", name="ps_o")

                def scores(tg, qa):
                    ps_s = pss.tile([128, 2, SC], F32, tag="pss", name="ps_s")
                    for i in range(2):
                        t = tg * 2 + i
                        nc.tensor.matmul(
                            ps_s[:, i, :],
                            lhsT=kt_sb[:, t * 128:(t + 1) * 128],
                            rhs=qa,
                            start=True, stop=True,
                        )
                    return ps_s

                q_ap = qt_sb[:, h, sc * SC:(sc + 1) * SC]
                pre_out = None
                cur = pre if pre is not None else scores(0, q_ap)
                for tg in range(KT_TILES // 2):
                    nxt = (scores(tg + 1, q_ap)
                           if tg < KT_TILES // 2 - 1 else None)
                    nc.scalar.activation(
                        pt[:, 2 * tg:2 * tg + 2, :], cur[:], AF.Exp, scale=SCALE
                    )
                    if tg == KT_TILES // 2 - 1 and nxt_combo is not None:
                        nsc, nh = nxt_combo
                        pre_out = scores(
                            0, qt_sb[:, nh, nsc * SC:(nsc + 1) * SC]
                        )
                    for i in range(2):
                        t = tg * 2 + i
                        nc.tensor.matmul(
                            ps_o[:],
                            lhsT=v_sb[:, t, :],
                            rhs=pt[:, t, :],
                            start=(t == 0),
                            stop=(t == KT_TILES - 1),
                            skip_group_check=True,
                        )
                    # denominator partials: gpsimd owns the EARLY tiles 0..7
                    # (slow ~1.2us/add, but its chain becomes eligible from
                    # exp(0) so it has the whole combo to finish); DVE owns
                    # the late tiles 8..15 (fast, finishes right after exp7).
                    # gpsimd runs ONLY tensor_add all kernel long — mixing op
                    # kinds forces a DSP LIBRARY_RELOAD each switch.  First op
                    # of each chain adds two pt tiles to avoid a copy.
                    if tg < 4:
                        if tg == 0:
                            nc.gpsimd.tensor_add(accG[:], pt[:, 0, :], pt[:, 1, :])
                        else:
                            for i in range(2):
                                t = tg * 2 + i
                                nc.gpsimd.tensor_add(accG[:], accG[:], pt[:, t, :])
                    else:
                        if tg == 4:
                            nc.vector.tensor_add(accD[:], pt[:, 8, :], pt[:, 9, :])
                        else:
                            for i in range(2):
                                t = tg * 2 + i
                                nc.vector.tensor_add(accD[:], accD[:], pt[:, t, :])
                    emit_filler(tg)
                    cur = nxt
                return ps_o, accD, accG, pre_out

            def tail(sc, h, ot_sb, ps_o, accD, accG):
                """denominator -> reciprocal -> normalize.  The all-ones
                128x128 stationary makes ones^T @ acc produce the column
                sums replicated on ALL partitions — denominator sum and
                partition-broadcast fused into one matmul pair."""
                ps_db = psm.tile([128, SC], F32, tag="psm", name="ps_db")
                nc.tensor.matmul(
                    ps_db[:], lhsT=ones_sq[:], rhs=accG[:],
                    start=True, stop=False, skip_group_check=True,
                )
                nc.tensor.matmul(
                    ps_db[:], lhsT=ones_sq[:], rhs=accD[:],
                    start=False, stop=True, skip_group_check=True,
                )
                bc = accp.tile([128, SC], F32, tag="bc", name="bc")
                nc.vector.reciprocal_approx_fast(bc[:], ps_db[:])
                nc.vector.tensor_mul(ot_sb[:, h, :], ps_o[:], bc[:])
                if h == R - 1:
                    op_q.extend(op_pending.pop(0))

            op_pending = []  # per-sc out-proj group lists, released by tail(sc,3)

            def queue_out_proj(sc, ot_sb):
                groups = []
                for st in range(SC // 128):
                    for oc in range(D // 512):
                        def go(st=st, oc=oc):
                            ps_f = psm.tile([128, 512], F32, tag="psm", name="ps_f")
                            for dv in range(R):
                                nc.tensor.matmul(
                                    ps_f[:],
                                    lhsT=ot_sb[:, dv, st * 128:(st + 1) * 128],
                                    rhs=wo_sb[:, dv, oc * 512:(oc + 1) * 512],
                                    start=(dv == 0),
                                    stop=(dv == R - 1),
                                    skip_group_check=True,
                                )
                            o_t = outp.tile([128, 512], BF16, tag="out", name="o_t")
                            nc.vector.tensor_copy(o_t[:], ps_f[:])
                            nc.sync.dma_start(
                                OUT.ap()[
                                    sc * SC + st * 128: sc * SC + (st + 1) * 128,
                                    oc * 512:(oc + 1) * 512,
                                ],
                                o_t[:],
                            )
                        groups.append(go)
                op_pending.append(groups)

            combos = [(sc, h) for sc in range(N_SC) for h in range(R)]
            pre = None
            ot_sb = None
            for idx, (sc, h) in enumerate(combos):
                if h == 0:
                    ot_sb = otp.tile([128, R, SC], BF16, tag="ot", name="ot_sb")
                nxt_combo = combos[idx + 1] if idx + 1 < len(combos) else None
                ps_o, accD, accG, pre = t_loop(sc, h, pre, nxt_combo)
                tail_q.append(
                    lambda sc=sc, h=h, ot_sb=ot_sb, ps_o=ps_o, accD=accD,
                    accG=accG: tail(sc, h, ot_sb, ps_o, accD, accG)
                )
                if h == R - 1:
                    queue_out_proj(sc, ot_sb)
            while tail_q:
                tail_q.pop(0)()
            while op_q:
                op_q.pop(0)()

    nc.compile()
    return nc


def _get_nc():
    if "nc" not in _CACHED:
        _CACHED["nc"] = _build()
    return _CACHED["nc"]


def _make_in_maps(x, Wq, bq, Wk, bk, Wv, bv, Wo):
    in_maps = []
    xts = [np.ascontiguousarray(x[b].T).astype(NPBF16) for b in range(2)]
    Wq_b = Wq.astype(NPBF16)
    Wk_b = Wk.astype(NPBF16)
    Wv_b = Wv.astype(NPBF16)
    Wo_b = Wo.astype(NPBF16)
    for core in range(8):
        b, g = divmod(core, 4)
        in_maps.append({
            "xt": xts[b],
            "wq": np.ascontiguousarray(Wq_b[:, g * GD:(g + 1) * GD]),
            "wk": np.ascontiguousarray(Wk_b[:, g * HD:(g + 1) * HD]),
            "wv": np.ascontiguousarray(Wv_b[:, g * HD:(g + 1) * HD]),
            "wo": np.ascontiguousarray(Wo_b[g * GD:(g + 1) * GD, :]),
            "bq": np.ascontiguousarray(
                bq[g * GD:(g + 1) * GD].reshape(R, 128).T
            ),
            "bk": bk[g * HD:(g + 1) * HD].reshape(HD, 1).copy(),
            "bv": bv[g * HD:(g + 1) * HD].reshape(HD, 1).copy(),
        })
    return in_maps


def kernel(x, Wq, bq, Wk, bk, Wv, bv, Wo, bo, _trace=False):
    x = np.asarray(x, dtype=np.float32)
    nc = _get_nc()
    in_maps = _make_in_maps(
        x,
        np.asarray(Wq, np.float32), np.asarray(bq, np.float32),
        np.asarray(Wk, np.float32), np.asarray(bk, np.float32),
        np.asarray(Wv, np.float32), np.asarray(bv, np.float32),
        np.asarray(Wo, np.float32),
    )
    res = run_bass_kernel_spmd(nc, in_maps, list(range(8)), trace=_trace)
    bo = np.asarray(bo, np.float32)
    out = np.empty((2, S, D), np.float32)
    for b in range(2):
        acc = res.results[b * 4]["out"].astype(np.float32)
        for g in range(1, 4):
            acc = acc + res.results[b * 4 + g]["out"].astype(np.float32)
        out[b] = acc + bo[None, :]
    if _trace:
        return out, res
    return out


# revision 24
# speedup vs baseline: 1.6636x; 1.0638x over previous
"""GQA attention block on 8 Trainium2 cores — all-bf16 matmul pipeline.

Sharding: data-parallel over batch B=2 x tensor-parallel over the 4 KV groups
(cores 0-3 -> batch 0 groups 0-3, cores 4-7 -> batch 1 groups 0-3).
Each core computes Q/K/V projections for its group, attention for its 4 query
heads, and a row-sharded partial of the output projection.  The host sums the
4 partials per batch (fp32) and adds the output bias.

v2 changes vs the fp32r baseline:
  - every matmul operand is bf16 (fp32 PSUM accumulation).  bf16 stationary
    weights get Fast Weight Load; the fp32r baseline measured 324 ns per
    128x128x512 matmul vs the 216 ns warm roofline, most of it the 4-byte
    weight-load path.
  - Q/K/V bias-adds moved from Scalar(ACT) to Vector (tensor_scalar_add) so
    ACT does nothing but the softmax EXPs (hard floor (N+352)/1.2 ns each).
  - softmax denominator partial sums split Vector/GpSimd, recip broadcast on
    GpSimd (partition_broadcast) instead of a PE ones-matmul + ACT copy.
  - softmax tails and the out-projection of chunk sc are spread as filler
    into the ACT-paced slots of the next chunk's score/attnV loops, keeping
    the PE stream dense.
  - OUT partials are bf16 (halves output DMA); host accumulates in fp32.
"""
import sys

sys.path.insert(0, "/opt/trn_rl_repo")

import math
from contextlib import ExitStack

import numpy as np
import ml_dtypes

import concourse.bacc as bacc
import concourse.tile as tile
import concourse.mybir as mybir
from concourse.bass_utils import run_bass_kernel_spmd
from concourse.masks import make_identity

F32 = mybir.dt.float32
BF16 = mybir.dt.bfloat16
AF = mybir.ActivationFunctionType
NPBF16 = ml_dtypes.bfloat16

D = 2048          # d_model
S = 2048          # sequence length
HD = 128          # head dim
R = 4             # q heads per kv group (on one core)
GD = R * HD       # 512: q-projection width per core
KT_TILES = S // 128   # 16 key-time tiles
KD_TILES = D // 128   # 16 contraction tiles for projections
N_SC = 4          # s-chunks of 512
SC = S // N_SC    # 512
SCALE = 1.0 / math.sqrt(HD)

_CACHED = {}


def _build():
    nc = bacc.Bacc("TRN2", target_bir_lowering=False, debug=False, num_devices=8)

    XT = nc.dram_tensor("xt", [D, S], BF16, kind="ExternalInput")
    WQ = nc.dram_tensor("wq", [D, GD], BF16, kind="ExternalInput")
    WK = nc.dram_tensor("wk", [D, HD], BF16, kind="ExternalInput")
    WV = nc.dram_tensor("wv", [D, HD], BF16, kind="ExternalInput")
    WO = nc.dram_tensor("wo", [GD, D], BF16, kind="ExternalInput")
    BQ = nc.dram_tensor("bq", [128, R], F32, kind="ExternalInput")
    BK = nc.dram_tensor("bk", [128, 1], F32, kind="ExternalInput")
    BV = nc.dram_tensor("bv", [128, 1], F32, kind="ExternalInput")
    OUT = nc.dram_tensor("out", [S, D], BF16, kind="ExternalOutput")

    with tile.TileContext(nc) as tc, ExitStack() as ctx:
        # ---- long-lived tiles ----
        lp = ctx.enter_context(tc.tile_pool(name="long", bufs=1))
        qt_sb = lp.tile([128, R, S], BF16)        # Q^T per head: [dq, h, s]
        kt_sb = lp.tile([128, S], BF16)           # K^T: [dk, t]
        vt_sb = lp.tile([128, S], BF16)           # V^T: [dv, t]
        v_sb = lp.tile([128, KT_TILES, HD], BF16) # V natural: [t_sub, t_tile, dv]
        bq_sb = lp.tile([128, R], F32)
        bk_sb = lp.tile([128, 1], F32)
        bv_sb = lp.tile([128, 1], F32)
        ones_sq = lp.tile([128, 128], BF16)
        ident = lp.tile([128, 128], BF16)

        nc.sync.dma_start(bq_sb[:], BQ.ap())
        nc.sync.dma_start(bk_sb[:], BK.ap())
        nc.sync.dma_start(bv_sb[:], BV.ap())

        nc.gpsimd.memset(ones_sq[:], 1.0)
        make_identity(nc, ident[:])

        # ---- phase A: projections ----
        with ExitStack() as actx:
            wp = actx.enter_context(tc.tile_pool(name="wqkv", bufs=1))
            xp = actx.enter_context(tc.tile_pool(name="xt", bufs=2))
            psa = actx.enter_context(tc.tile_pool(name="psa", bufs=4, space="PSUM"))

            wq_sb = wp.tile([128, KD_TILES, GD], BF16)
            wk_sb = wp.tile([128, KD_TILES, HD], BF16)
            wv_sb = wp.tile([128, KD_TILES, HD], BF16)
            wq_r = WQ.ap().rearrange("(ko p) n -> p ko n", p=128)
            wk_r = WK.ap().rearrange("(ko p) n -> p ko n", p=128)
            wv_r = WV.ap().rearrange("(ko p) n -> p ko n", p=128)

            for sc in range(N_SC):
                xt = xp.tile([128, KD_TILES, SC], BF16, tag="xt")
                xt_r = XT.ap()[:, sc * SC:(sc + 1) * SC].rearrange(
                    "(ko p) s -> p ko s", p=128
                )
                # interleave per-k so the k=0 matmuls can start almost
                # immediately (weights ride along with the first chunk);
                # wk/wv/ arrive while the Q matmuls run.
                for k in range(KD_TILES):
                    nc.sync.dma_start(xt[:, k, :], xt_r[:, k, :])
                    if sc == 0:
                        nc.sync.dma_start(wq_sb[:, k, :], wq_r[:, k, :])
                if sc == 0:
                    for k in range(KD_TILES):
                        nc.sync.dma_start(wk_sb[:, k, :], wk_r[:, k, :])
                        nc.sync.dma_start(wv_sb[:, k, :], wv_r[:, k, :])
                # Q^T for the 4 heads, k-outer so each weight/x chunk is
                # reused by 4 matmuls as soon as it lands (keeps the start
                # of phase A PE-paced, not DMA-paced)
                # psa0/psa1 are reused by K/V right after the Q heads, so they
                # get 2 buffers (frees the PE from waiting on the DVE bias
                # chain); psa2/psa3 single.  4*1 + 2*... = 6 banks + pst 2 = 8.
                ps_q = [psa.tile([128, SC], F32, tag=f"psa{dq}",
                                 bufs=(2 if dq < 2 else 1),
                                 name=f"ps_q{dq}") for dq in range(R)]
                for k in range(KD_TILES):
                    for dq in range(R):
                        nc.tensor.matmul(
                            ps_q[dq][:],
                            lhsT=wq_sb[:, k, dq * 128:(dq + 1) * 128],
                            rhs=xt[:, k, :],
                            start=(k == 0),
                            stop=(k == KD_TILES - 1),
                            skip_group_check=True,
                        )
                for dq in range(R):
                    nc.vector.tensor_scalar_add(
                        qt_sb[:, dq, sc * SC:(sc + 1) * SC], ps_q[dq][:],
                        bq_sb[:, dq:dq + 1],
                    )
                # K^T and V^T, k-outer
                ps_k = psa.tile([128, SC], F32, tag="psa0", bufs=2, name="ps_k")
                ps_v = psa.tile([128, SC], F32, tag="psa1", bufs=2, name="ps_v")
                for k in range(KD_TILES):
                    nc.tensor.matmul(
                        ps_k[:], lhsT=wk_sb[:, k, :], rhs=xt[:, k, :],
                        start=(k == 0), stop=(k == KD_TILES - 1),
                        skip_group_check=True,
                    )
                    nc.tensor.matmul(
                        ps_v[:], lhsT=wv_sb[:, k, :], rhs=xt[:, k, :],
                        start=(k == 0), stop=(k == KD_TILES - 1),
                        skip_group_check=True,
                    )
                nc.vector.tensor_scalar_add(
                    kt_sb[:, sc * SC:(sc + 1) * SC], ps_k[:], bk_sb[:],
                )
                nc.vector.tensor_scalar_add(
                    vt_sb[:, sc * SC:(sc + 1) * SC], ps_v[:], bv_sb[:],
                )

            # V^T -> V natural (16 PE transposes)
            pst = actx.enter_context(tc.tile_pool(name="pst", bufs=2, space="PSUM"))
            for t in range(KT_TILES):
                pt_ps = pst.tile([128, 128], BF16, tag="pst")
                nc.tensor.transpose(
                    pt_ps[:], vt_sb[:, t * 128:(t + 1) * 128], ident[:]
                )
                nc.vector.tensor_copy(v_sb[:, t, :], pt_ps[:])

        # ---- phase B: attention + out-proj ----
        with ExitStack() as bctx:
            wop = bctx.enter_context(tc.tile_pool(name="wo", bufs=1))
            wo_sb = wop.tile([128, R, D], BF16)
            nc.sync.dma_start(wo_sb[:], WO.ap().rearrange("(h p) n -> p h n", p=128))

            pss = bctx.enter_context(tc.tile_pool(name="pss", bufs=2, space="PSUM"))
            pso = bctx.enter_context(tc.tile_pool(name="pso", bufs=2, space="PSUM"))
            psm = bctx.enter_context(tc.tile_pool(name="psm", bufs=2, space="PSUM"))
            ptp = bctx.enter_context(tc.tile_pool(name="ptp", bufs=2))
            accp = bctx.enter_context(tc.tile_pool(name="accp", bufs=2))
            otp = bctx.enter_context(tc.tile_pool(name="otp", bufs=2))
            outp = bctx.enter_context(tc.tile_pool(name="outp", bufs=4))

            # filler queues consumed inside t_loop slots.  The tail of combo
            # i is emitted at combo i+1's tg==5 slot (late enough that the
            # slow gpsimd denominator chain of combo i has finished, so the
            # PE's ps_d matmul never stalls the in-order PE stream).
            # out-proj groups of chunk sc are gated until tail(sc,3) emitted.
            tail_q = []
            op_q = []

            def emit_filler(tg):
                if tg == 5 and tail_q:
                    tail_q.pop(0)()
                elif tg != 5 and op_q:
                    op_q.pop(0)()

            def t_loop(sc, h, pre, nxt_combo):
                """scores -> exp -> attnV accumulation + split denom sums.
                `pre` is this combo's first score pair (pre-emitted by the
                previous combo so the PE never drains at a combo boundary);
                returns the next combo's pre-emitted pair."""
                pt = ptp.tile([128, KT_TILES, SC], BF16, tag="pt", name="pt")
                accD = accp.tile([128, SC], BF16, tag="accD", name="accD")
                accG = accp.tile([128, SC], BF16, tag="accG", name="accG")
                ps_o = pso.tile([128, SC], F32, tag="pso", name="ps_o")

                def scores(tg, qa):
                    ps_s = pss.tile([128, 2, SC], F32, tag="pss", name="ps_s")
                    for i in range(2):
                        t = tg * 2 + i
                        nc.tensor.matmul(
                            ps_s[:, i, :],
                            lhsT=kt_sb[:, t * 128:(t + 1) * 128],
                            rhs=qa,
                            start=True, stop=True,
                        )
                    return ps_s

                q_ap = qt_sb[:, h, sc * SC:(sc + 1) * SC]
                pre_out = None
                cur = pre if pre is not None else scores(0, q_ap)
                for tg in range(KT_TILES // 2):
                    nxt = (scores(tg + 1, q_ap)
                           if tg < KT_TILES // 2 - 1 else None)
                    nc.scalar.activation(
                        pt[:, 2 * tg:2 * tg + 2, :], cur[:], AF.Exp, scale=SCALE
                    )
                    if tg == KT_TILES // 2 - 1 and nxt_combo is not None:
                        nsc, nh = nxt_combo
                        pre_out = scores(
                            0, qt_sb[:, nh, nsc * SC:(nsc + 1) * SC]
                        )
                    for i in range(2):
                        t = tg * 2 + i
                        nc.tensor.matmul(
                            ps_o[:],
                            lhsT=v_sb[:, t, :],
                            rhs=pt[:, t, :],
                            start=(t == 0),
                            stop=(t == KT_TILES - 1),
                            skip_group_check=True,
                        )
                    # denominator partials, balanced by measured rates (DVE
                    # add 0.64us, gpsimd add 1.19us, PE MM 0.22us): gpsimd
                    # owns the EARLY tiles 0..4 (eligible from exp0, done by
                    # mid-combo), DVE tiles 5..13, and tiles 14/15 are summed
                    # by the PE ones-matmul directly in the tail (no chain).
                    # gpsimd runs ONLY tensor_add all kernel long — mixing op
                    # kinds forces a DSP LIBRARY_RELOAD each switch.  First op
                    # of each chain adds two pt tiles to avoid a copy.
                    for i in range(2):
                        t = tg * 2 + i
                        if t == 0:
                            nc.gpsimd.tensor_add(accG[:], pt[:, 0, :], pt[:, 1, :])
                        elif t == 1:
                            pass  # consumed by t==0
                        elif t < 5:
                            nc.gpsimd.tensor_add(accG[:], accG[:], pt[:, t, :])
                        elif t == 5:
                            pass  # consumed at t==6 (pt6 exists only then)
                        elif t == 6:
                            nc.vector.tensor_add(accD[:], pt[:, 5, :], pt[:, 6, :])
                        elif t < 14:
                            nc.vector.tensor_add(accD[:], accD[:], pt[:, t, :])
                        # t == 14, 15: summed by the PE directly in tail()
                    emit_filler(tg)
                    cur = nxt
                return ps_o, accD, accG, pt, pre_out

            def tail(sc, h, ot_sb, ps_o, accD, accG, pt):
                """denominator -> reciprocal -> normalize.  The all-ones
                128x128 stationary makes ones^T @ acc produce the column
                sums replicated on ALL partitions — denominator sum and
                partition-broadcast fused into one matmul accumulation
                (partial chains + the two rawest exp tiles directly)."""
                ps_db = psm.tile([128, SC], F32, tag="psm", name="ps_db")
                for j, rhs in enumerate(
                    (accG[:], accD[:], pt[:, 14, :], pt[:, 15, :])
                ):
                    nc.tensor.matmul(
                        ps_db[:], lhsT=ones_sq[:], rhs=rhs,
                        start=(j == 0), stop=(j == 3), skip_group_check=True,
                    )
                bc = accp.tile([128, SC], F32, tag="bc", name="bc")
                nc.vector.reciprocal_approx_fast(bc[:], ps_db[:])
                nc.vector.tensor_mul(ot_sb[:, h, :], ps_o[:], bc[:])
                if h == R - 1:
                    op_q.extend(op_pending.pop(0))

            op_pending = []  # per-sc out-proj group lists, released by tail(sc,3)

            def queue_out_proj(sc, ot_sb):
                groups = []
                for st in range(SC // 128):
                    for oc in range(D // 512):
                        def go(st=st, oc=oc):
                            ps_f = psm.tile([128, 512], F32, tag="psm", name="ps_f")
                            for dv in range(R):
                                nc.tensor.matmul(
                                    ps_f[:],
                                    lhsT=ot_sb[:, dv, st * 128:(st + 1) * 128],
                                    rhs=wo_sb[:, dv, oc * 512:(oc + 1) * 512],
                                    start=(dv == 0),
                                    stop=(dv == R - 1),
                                    skip_group_check=True,
                                )
                            o_t = outp.tile([128, 512], BF16, tag="out", name="o_t")
                            nc.vector.tensor_copy(o_t[:], ps_f[:])
                            nc.sync.dma_start(
                                OUT.ap()[
                                    sc * SC + st * 128: sc * SC + (st + 1) * 128,
                                    oc * 512:(oc + 1) * 512,
                                ],
                                o_t[:],
                            )
                        groups.append(go)
                op_pending.append(groups)

            combos = [(sc, h) for sc in range(N_SC) for h in range(R)]
            pre = None
            ot_sb = None
            for idx, (sc, h) in enumerate(combos):
                if h == 0:
                    ot_sb = otp.tile([128, R, SC], BF16, tag="ot", name="ot_sb")
                nxt_combo = combos[idx + 1] if idx + 1 < len(combos) else None
                ps_o, accD, accG, pt_t, pre = t_loop(sc, h, pre, nxt_combo)
                tail_q.append(
                    lambda sc=sc, h=h, ot_sb=ot_sb, ps_o=ps_o, accD=accD,
                    accG=accG, pt_t=pt_t: tail(sc, h, ot_sb, ps_o, accD, accG,
                                               pt_t)
                )
                if h == R - 1:
                    queue_out_proj(sc, ot_sb)
            while tail_q:
                tail_q.pop(0)()
            while op_q:
                op_q.pop(0)()

    nc.compile()
    return nc


def _get_nc():
    if "nc" not in _CACHED:
        _CACHED["nc"] = _build()
    return _CACHED["nc"]


def _make_in_maps(x, Wq, bq, Wk, bk, Wv, bv, Wo):
    in_maps = []
    xts = [np.ascontiguousarray(x[b].T).astype(NPBF16) for b in range(2)]
    Wq_b = Wq.astype(NPBF16)
    Wk_b = Wk.astype(NPBF16)
    Wv_b = Wv.astype(NPBF16)
    Wo_b = Wo.astype(NPBF16)
    for core in range(8):
        b, g = divmod(core, 4)
        in_maps.append({
            "xt": xts[b],
            "wq": np.ascontiguousarray(Wq_b[:, g * GD:(g + 1) * GD]),
            "wk": np.ascontiguousarray(Wk_b[:, g * HD:(g + 1) * HD]),
            "wv": np.ascontiguousarray(Wv_b[:, g * HD:(g + 1) * HD]),
            "wo": np.ascontiguousarray(Wo_b[g * GD:(g + 1) * GD, :]),
            "bq": np.ascontiguousarray(
                bq[g * GD:(g + 1) * GD].reshape(R, 128).T
            ),
            "bk": bk[g * HD:(g + 1) * HD].reshape(HD, 1).copy(),
            "bv": bv[g * HD:(g + 1) * HD].reshape(HD, 1).copy(),
        })
    return in_maps


def kernel(x, Wq, bq, Wk, bk, Wv, bv, Wo, bo, _trace=False):
    x = np.asarray(x, dtype=np.float32)
    nc = _get_nc()
    in_maps = _make_in_maps(
        x,
        np.asarray(Wq, np.float32), np.asarray(bq, np.float32),
        np.asarray(Wk, np.float32), np.asarray(bk, np.float32),
        np.asarray(Wv, np.float32), np.asarray(bv, np.float32),
        np.asarray(Wo, np.float32),
    )
    res = run_bass_kernel_spmd(nc, in_maps, list(range(8)), trace=_trace)
    bo = np.asarray(bo, np.float32)
    out = np.empty((2, S, D), np.float32)
    for b in range(2):
        acc = res.results[b * 4]["out"].astype(np.float32)
        for g in range(1, 4):
            acc = acc + res.results[b * 4 + g]["out"].astype(np.float32)
        out[b] = acc + bo[None, :]
    if _trace:
        return out, res
    return out


# revision 30
# speedup vs baseline: 1.7820x; 1.0712x over previous
"""GQA attention block on 8 Trainium2 cores — all-bf16 matmul pipeline.

Sharding: data-parallel over batch B=2 x tensor-parallel over the 4 KV groups
(cores 0-3 -> batch 0 groups 0-3, cores 4-7 -> batch 1 groups 0-3).
Each core computes Q/K/V projections for its group, attention for its 4 query
heads, and a row-sharded partial of the output projection.  The host sums the
4 partials per batch (fp32) and adds the output bias.

v2 changes vs the fp32r baseline:
  - every matmul operand is bf16 (fp32 PSUM accumulation).  bf16 stationary
    weights get Fast Weight Load; the fp32r baseline measured 324 ns per
    128x128x512 matmul vs the 216 ns warm roofline, most of it the 4-byte
    weight-load path.
  - Q/K/V bias-adds moved from Scalar(ACT) to Vector (tensor_scalar_add) so
    ACT does nothing but the softmax EXPs (hard floor (N+352)/1.2 ns each).
  - softmax denominator partial sums split Vector/GpSimd, recip broadcast on
    GpSimd (partition_broadcast) instead of a PE ones-matmul + ACT copy.
  - softmax tails and the out-projection of chunk sc are spread as filler
    into the ACT-paced slots of the next chunk's score/attnV loops, keeping
    the PE stream dense.
  - OUT partials are bf16 (halves output DMA); host accumulates in fp32.
"""
import sys

sys.path.insert(0, "/opt/trn_rl_repo")

import math
from contextlib import ExitStack

import numpy as np
import ml_dtypes

import concourse.bacc as bacc
import concourse.tile as tile
import concourse.mybir as mybir
from concourse.bass_utils import run_bass_kernel_spmd
from concourse.masks import make_identity

F32 = mybir.dt.float32
BF16 = mybir.dt.bfloat16
AF = mybir.ActivationFunctionType
NPBF16 = ml_dtypes.bfloat16

D = 2048          # d_model
S = 2048          # sequence length
HD = 128          # head dim
R = 4             # q heads per kv group (on one core)
GD = R * HD       # 512: q-projection width per core
KT_TILES = S // 128   # 16 key-time tiles
KD_TILES = D // 128   # 16 contraction tiles for projections
N_SC = 4          # s-chunks of 512
SC = S // N_SC    # 512
SCALE = 1.0 / math.sqrt(HD)

_CACHED = {}


def _build():
    nc = bacc.Bacc("TRN2", target_bir_lowering=False, debug=False, num_devices=8)

    # inputs are pre-tiled on the host so every DMA reads CONTIGUOUS
    # multi-KB runs per partition (a [128,512] tile of a row-major [D,S]
    # matrix costs 128 separate 1KB descriptors; packed, a 4-chunk group
    # is 128 x 4KB)
    XT = nc.dram_tensor("xt", [128, N_SC, KD_TILES, SC], BF16,
                        kind="ExternalInput")
    WQ = nc.dram_tensor("wq", [128, KD_TILES, GD], BF16, kind="ExternalInput")
    WK = nc.dram_tensor("wk", [128, KD_TILES, HD], BF16, kind="ExternalInput")
    WV = nc.dram_tensor("wv", [128, KD_TILES, HD], BF16, kind="ExternalInput")
    WO = nc.dram_tensor("wo", [128, R, D], BF16, kind="ExternalInput")
    BQ = nc.dram_tensor("bq", [128, R], F32, kind="ExternalInput")
    BK = nc.dram_tensor("bk", [128, 1], F32, kind="ExternalInput")
    BV = nc.dram_tensor("bv", [128, 1], F32, kind="ExternalInput")
    OUT = nc.dram_tensor("out", [S, D], BF16, kind="ExternalOutput")

    with tile.TileContext(nc) as tc, ExitStack() as ctx:
        # ---- long-lived tiles ----
        lp = ctx.enter_context(tc.tile_pool(name="long", bufs=1))
        qt_sb = lp.tile([128, R, S], BF16)        # Q^T per head: [dq, h, s]
        kt_sb = lp.tile([128, S], BF16)           # K^T: [dk, t]
        vt_sb = lp.tile([128, S], BF16)           # V^T: [dv, t]
        v_sb = lp.tile([128, KT_TILES, HD], BF16) # V natural: [t_sub, t_tile, dv]
        bq_sb = lp.tile([128, R], F32)
        bk_sb = lp.tile([128, 1], F32)
        bv_sb = lp.tile([128, 1], F32)
        ones_sq = lp.tile([128, 128], BF16)
        ident = lp.tile([128, 128], BF16)

        nc.sync.dma_start(bq_sb[:], BQ.ap())
        nc.sync.dma_start(bk_sb[:], BK.ap())
        nc.sync.dma_start(bv_sb[:], BV.ap())

        nc.gpsimd.memset(ones_sq[:], 1.0)
        make_identity(nc, ident[:])

        # ---- phase A: projections ----
        with ExitStack() as actx:
            wp = actx.enter_context(tc.tile_pool(name="wqkv", bufs=1))
            xp = actx.enter_context(tc.tile_pool(name="xt", bufs=2))
            psa = actx.enter_context(tc.tile_pool(name="psa", bufs=4, space="PSUM"))

            wq_sb = wp.tile([128, KD_TILES, GD], BF16)
            wk_sb = wp.tile([128, KD_TILES, HD], BF16)
            wv_sb = wp.tile([128, KD_TILES, HD], BF16)

            for sc in range(N_SC):
                xt = xp.tile([128, KD_TILES, SC], BF16, tag="xt")
                # 4-chunk groups: x and wq interleaved so the k=0 matmuls
                # start almost immediately; wk/wv arrive while Q runs.
                for g in range(0, KD_TILES, 4):
                    nc.sync.dma_start(xt[:, g:g + 4, :], XT.ap()[:, sc, g:g + 4, :])
                    if sc == 0:
                        nc.sync.dma_start(wq_sb[:, g:g + 4, :], WQ.ap()[:, g:g + 4, :])
                if sc == 0:
                    nc.sync.dma_start(wk_sb[:], WK.ap())
                    nc.sync.dma_start(wv_sb[:], WV.ap())
                # Q^T for the 4 heads, k-outer so each weight/x chunk is
                # reused by 4 matmuls as soon as it lands (keeps the start
                # of phase A PE-paced, not DMA-paced)
                # psa0/psa1 are reused by K/V right after the Q heads, so they
                # get 2 buffers (frees the PE from waiting on the DVE bias
                # chain); psa2/psa3 single.  4*1 + 2*... = 6 banks + pst 2 = 8.
                ps_q = [psa.tile([128, SC], F32, tag=f"psa{dq}",
                                 bufs=(2 if dq < 2 else 1),
                                 name=f"ps_q{dq}") for dq in range(R)]
                for k in range(KD_TILES):
                    for dq in range(R):
                        nc.tensor.matmul(
                            ps_q[dq][:],
                            lhsT=wq_sb[:, k, dq * 128:(dq + 1) * 128],
                            rhs=xt[:, k, :],
                            start=(k == 0),
                            stop=(k == KD_TILES - 1),
                            skip_group_check=True,
                        )
                for dq in range(R):
                    nc.vector.tensor_scalar_add(
                        qt_sb[:, dq, sc * SC:(sc + 1) * SC], ps_q[dq][:],
                        bq_sb[:, dq:dq + 1],
                    )
                # K^T and V^T, k-outer
                ps_k = psa.tile([128, SC], F32, tag="psa0", bufs=2, name="ps_k")
                ps_v = psa.tile([128, SC], F32, tag="psa1", bufs=2, name="ps_v")
                for k in range(KD_TILES):
                    nc.tensor.matmul(
                        ps_k[:], lhsT=wk_sb[:, k, :], rhs=xt[:, k, :],
                        start=(k == 0), stop=(k == KD_TILES - 1),
                        skip_group_check=True,
                    )
                    nc.tensor.matmul(
                        ps_v[:], lhsT=wv_sb[:, k, :], rhs=xt[:, k, :],
                        start=(k == 0), stop=(k == KD_TILES - 1),
                        skip_group_check=True,
                    )
                nc.vector.tensor_scalar_add(
                    kt_sb[:, sc * SC:(sc + 1) * SC], ps_k[:], bk_sb[:],
                )
                nc.vector.tensor_scalar_add(
                    vt_sb[:, sc * SC:(sc + 1) * SC], ps_v[:], bv_sb[:],
                )

            # V^T -> V natural (16 PE transposes)
            pst = actx.enter_context(tc.tile_pool(name="pst", bufs=2, space="PSUM"))
            for t in range(KT_TILES):
                pt_ps = pst.tile([128, 128], BF16, tag="pst")
                nc.tensor.transpose(
                    pt_ps[:], vt_sb[:, t * 128:(t + 1) * 128], ident[:]
                )
                nc.vector.tensor_copy(v_sb[:, t, :], pt_ps[:])

        # ---- phase B: attention + out-proj ----
        with ExitStack() as bctx:
            wop = bctx.enter_context(tc.tile_pool(name="wo", bufs=1))
            wo_sb = wop.tile([128, R, D], BF16)
            nc.sync.dma_start(wo_sb[:], WO.ap())

            pss = bctx.enter_context(tc.tile_pool(name="pss", bufs=2, space="PSUM"))
            pso = bctx.enter_context(tc.tile_pool(name="pso", bufs=2, space="PSUM"))
            psm = bctx.enter_context(tc.tile_pool(name="psm", bufs=2, space="PSUM"))
            ptp = bctx.enter_context(tc.tile_pool(name="ptp", bufs=2))
            accp = bctx.enter_context(tc.tile_pool(name="accp", bufs=2))
            otp = bctx.enter_context(tc.tile_pool(name="otp", bufs=2))
            outp = bctx.enter_context(tc.tile_pool(name="outp", bufs=4))

            # filler queues consumed inside t_loop slots.  The tail of combo
            # i is emitted at combo i+1's tg==5 slot (late enough that the
            # slow gpsimd denominator chain of combo i has finished, so the
            # PE's ps_d matmul never stalls the in-order PE stream).
            # out-proj groups of chunk sc are gated until tail(sc,3) emitted.
            tail_q = []
            op_q = []
            op_budget = [0]  # per-combo cap, reset by t_loop

            def emit_filler(tg):
                if tg == 5 and tail_q:
                    tail_q.pop(0)()
                elif tg != 5 and op_q and op_budget[0] > 0:
                    op_budget[0] -= 1
                    op_q.pop(0)()

            def t_loop(sc, h, pre, nxt_combo):
                """scores -> exp -> attnV accumulation + split denom sums.
                `pre` is this combo's first score pair (pre-emitted by the
                previous combo so the PE never drains at a combo boundary);
                returns the next combo's pre-emitted pair."""
                pt = ptp.tile([128, KT_TILES, SC], BF16, tag="pt", name="pt")
                accD = accp.tile([128, SC], BF16, tag="accD", name="accD")
                accG = accp.tile([128, SC], BF16, tag="accG", name="accG")
                ps_o = pso.tile([128, SC], F32, tag="pso", name="ps_o")
                op_budget[0] = 4  # spread out-proj fillers evenly over combos

                def scores(tg, qa):
                    ps_s = pss.tile([128, 2, SC], F32, tag="pss", name="ps_s")
                    for i in range(2):
                        t = tg * 2 + i
                        nc.tensor.matmul(
                            ps_s[:, i, :],
                            lhsT=kt_sb[:, t * 128:(t + 1) * 128],
                            rhs=qa,
                            start=True, stop=True,
                        )
                    return ps_s

                q_ap = qt_sb[:, h, sc * SC:(sc + 1) * SC]
                pre_out = None
                cur = pre if pre is not None else scores(0, q_ap)
                for tg in range(KT_TILES // 2):
                    nxt = (scores(tg + 1, q_ap)
                           if tg < KT_TILES // 2 - 1 else None)
                    nc.scalar.activation(
                        pt[:, 2 * tg:2 * tg + 2, :], cur[:], AF.Exp, scale=SCALE
                    )
                    if tg == KT_TILES // 2 - 1 and nxt_combo is not None:
                        nsc, nh = nxt_combo
                        pre_out = scores(
                            0, qt_sb[:, nh, nsc * SC:(nsc + 1) * SC]
                        )
                    for i in range(2):
                        t = tg * 2 + i
                        nc.tensor.matmul(
                            ps_o[:],
                            lhsT=v_sb[:, t, :],
                            rhs=pt[:, t, :],
                            start=(t == 0),
                            stop=(t == KT_TILES - 1),
                            skip_group_check=True,
                        )
                    # denominator partials, balanced by measured rates (DVE
                    # add 0.64us, gpsimd add 1.19us, PE MM 0.22us): gpsimd
                    # owns the EARLY tiles 0..4 (eligible from exp0, done by
                    # mid-combo), DVE tiles 5..13, and tiles 14/15 are summed
                    # by the PE ones-matmul directly in the tail (no chain).
                    # gpsimd runs ONLY tensor_add all kernel long — mixing op
                    # kinds forces a DSP LIBRARY_RELOAD each switch.  First op
                    # of each chain adds two pt tiles to avoid a copy.
                    for i in range(2):
                        t = tg * 2 + i
                        if t == 0:
                            nc.gpsimd.tensor_add(accG[:], pt[:, 0, :], pt[:, 1, :])
                        elif t == 1:
                            pass  # consumed by t==0
                        elif t < 5:
                            nc.gpsimd.tensor_add(accG[:], accG[:], pt[:, t, :])
                        elif t == 5:
                            pass  # consumed at t==6 (pt6 exists only then)
                        elif t == 6:
                            nc.vector.tensor_add(accD[:], pt[:, 5, :], pt[:, 6, :])
                        elif t < 14:
                            nc.vector.tensor_add(accD[:], accD[:], pt[:, t, :])
                        # t == 14, 15: summed by the PE directly in tail()
                    emit_filler(tg)
                    cur = nxt
                return ps_o, accD, accG, pt, pre_out

            def tail(sc, h, ot_sb, ps_o, accD, accG, pt):
                """denominator -> reciprocal -> normalize.  The all-ones
                128x128 stationary makes ones^T @ acc produce the column
                sums replicated on ALL partitions — denominator sum and
                partition-broadcast fused into one matmul accumulation
                (partial chains + the two rawest exp tiles directly)."""
                ps_db = psm.tile([128, SC], F32, tag="psm", name="ps_db")
                for j, rhs in enumerate(
                    (accG[:], accD[:], pt[:, 14, :], pt[:, 15, :])
                ):
                    nc.tensor.matmul(
                        ps_db[:], lhsT=ones_sq[:], rhs=rhs,
                        start=(j == 0), stop=(j == 3), skip_group_check=True,
                    )
                bc = accp.tile([128, SC], F32, tag="bc", name="bc")
                nc.vector.reciprocal_approx_fast(bc[:], ps_db[:])
                nc.vector.tensor_mul(ot_sb[:, h, :], ps_o[:], bc[:])
                if h == R - 1:
                    op_q.extend(op_pending.pop(0))

            op_pending = []  # per-sc out-proj group lists, released by tail(sc,3)

            def queue_out_proj(sc, ot_sb):
                groups = []
                for st in range(SC // 128):
                    for oc in range(D // 512):
                        def go(st=st, oc=oc):
                            ps_f = psm.tile([128, 512], F32, tag="psm", name="ps_f")
                            for dv in range(R):
                                nc.tensor.matmul(
                                    ps_f[:],
                                    lhsT=ot_sb[:, dv, st * 128:(st + 1) * 128],
                                    rhs=wo_sb[:, dv, oc * 512:(oc + 1) * 512],
                                    start=(dv == 0),
                                    stop=(dv == R - 1),
                                    skip_group_check=True,
                                )
                            o_t = outp.tile([128, 512], BF16, tag="out", name="o_t")
                            nc.vector.tensor_copy(o_t[:], ps_f[:])
                            nc.sync.dma_start(
                                OUT.ap()[
                                    sc * SC + st * 128: sc * SC + (st + 1) * 128,
                                    oc * 512:(oc + 1) * 512,
                                ],
                                o_t[:],
                            )
                        groups.append(go)
                op_pending.append(groups)

            combos = [(sc, h) for sc in range(N_SC) for h in range(R)]
            pre = None
            ot_sb = None
            for idx, (sc, h) in enumerate(combos):
                if h == 0:
                    ot_sb = otp.tile([128, R, SC], BF16, tag="ot", name="ot_sb")
                nxt_combo = combos[idx + 1] if idx + 1 < len(combos) else None
                ps_o, accD, accG, pt_t, pre = t_loop(sc, h, pre, nxt_combo)
                tail_q.append(
                    lambda sc=sc, h=h, ot_sb=ot_sb, ps_o=ps_o, accD=accD,
                    accG=accG, pt_t=pt_t: tail(sc, h, ot_sb, ps_o, accD, accG,
                                               pt_t)
                )
                if h == R - 1:
                    queue_out_proj(sc, ot_sb)
            while tail_q:
                tail_q.pop(0)()
            while op_q:
                op_q.pop(0)()

    nc.compile()
    return nc


def _get_nc():
    if "nc" not in _CACHED:
        _CACHED["nc"] = _build()
    return _CACHED["nc"]


def _pack_kp(w):
    """[D, N] -> [128, KD_TILES, N]: row d = ko*128+p goes to [p, ko, :]."""
    n = w.shape[1]
    return np.ascontiguousarray(
        w.reshape(KD_TILES, 128, n).transpose(1, 0, 2)
    )


def _make_in_maps(x, Wq, bq, Wk, bk, Wv, bv, Wo):
    in_maps = []
    # xT [D,S] -> [p, sc, ko, s']  (per-partition-contiguous k-groups)
    xts = [
        np.ascontiguousarray(
            x[b].T.reshape(KD_TILES, 128, N_SC, SC).transpose(1, 2, 0, 3)
        ).astype(NPBF16)
        for b in range(2)
    ]
    Wq_b = Wq.astype(NPBF16)
    Wk_b = Wk.astype(NPBF16)
    Wv_b = Wv.astype(NPBF16)
    Wo_b = Wo.astype(NPBF16)
    for core in range(8):
        b, g = divmod(core, 4)
        wo_s = Wo_b[g * GD:(g + 1) * GD, :]
        in_maps.append({
            "xt": xts[b],
            "wq": _pack_kp(Wq_b[:, g * GD:(g + 1) * GD]),
            "wk": _pack_kp(Wk_b[:, g * HD:(g + 1) * HD]),
            "wv": _pack_kp(Wv_b[:, g * HD:(g + 1) * HD]),
            "wo": np.ascontiguousarray(
                wo_s.reshape(R, 128, D).transpose(1, 0, 2)
            ),
            "bq": np.ascontiguousarray(
                bq[g * GD:(g + 1) * GD].reshape(R, 128).T
            ),
            "bk": bk[g * HD:(g + 1) * HD].reshape(HD, 1).copy(),
            "bv": bv[g * HD:(g + 1) * HD].reshape(HD, 1).copy(),
        })
    return in_maps


def kernel(x, Wq, bq, Wk, bk, Wv, bv, Wo, bo, _trace=False):
    x = np.asarray(x, dtype=np.float32)
    nc = _get_nc()
    in_maps = _make_in_maps(
        x,
        np.asarray(Wq, np.float32), np.asarray(bq, np.float32),
        np.asarray(Wk, np.float32), np.asarray(bk, np.float32),
        np.asarray(Wv, np.float32), np.asarray(bv, np.float32),
        np.asarray(Wo, np.float32),
    )
    res = run_bass_kernel_spmd(nc, in_maps, list(range(8)), trace=_trace)
    bo = np.asarray(bo, np.float32)
    out = np.empty((2, S, D), np.float32)
    for b in range(2):
        acc = res.results[b * 4]["out"].astype(np.float32)
        for g in range(1, 4):
            acc = acc + res.results[b * 4 + g]["out"].astype(np.float32)
        out[b] = acc + bo[None, :]
    if _trace:
        return out, res
    return out
